# revision 1
# baseline (speedup 1.0000x reference)
"""Causal multi-head self-attention on 8 Trainium2 NeuronCores.

Problem: x[2,2048,1024], 16 heads, dk=64, causal softmax, fp32 in/out.

Sharding (data + tensor parallel, per the hint): core c handles batch
b = c//4 and head group g = c%4 (4 heads = 256 feature cols). wq/wk/wv
are column-sharded, wo row-sharded; each core returns a [D, S] partial
of out^T for its batch, and the host sums the 4 partials per batch.

Per-core kernel (layouts chosen so no on-device transposes are needed;
all matmul inputs bf16, accumulation fp32 in PSUM):
  - host supplies x^T [D, S] bf16; q^T/k^T [256, S] = w^T @ x^T on PE,
    v [S, 256] natural; v stored with a ones column per head (65-wide
    groups) so the AV matmul also produces softmax denominators.
  - scores^T tile [k=128, q<=1024] = k_h^T.T @ q_h^T, causal tiles
    only. Head pairs sit at partition bases 0/64 of the same tiles, so
    their K=64 matmuls row-tile and run concurrently on the PE array.
    The diagonal 128x128 block gets a staircase additive mask from one
    extra bf16 matmul: -240*(k-q) for k>q, 0 otherwise.
  - exp on ScalarE (scale=1/8 fused; no max-subtraction: |scores|<~3,
    masked entries underflow to exactly 0), psum -> bf16 sbuf.
  - av[65+, q] += v_aug.T @ e accumulated over k tiles (v head groups
    padded to 128 cols so weight loads take the fast path); row 64 =
    softmax denominator. vector reciprocal on row 64, broadcast across
    partitions by bouncing the row through DRAM (DMA cannot read a
    step-0 partition AP from SBUF; the gpsimd partition_broadcast and
    custom-DVE reciprocal ucodes proved broken through this runtime
    path), one tensor_mul normalizes into attnT [256, S] bf16. Odd
    heads bounce via SBUF tmp + DMA (compute engines cannot cross
    partition lanes).
  - out^T [D, S] fp32 = wo.T @ attnT on PE, evacuated on the vector
    engine, DMA'd out. Host sums the 4 partials per batch in fp64.
"""

import os
import sys

import numpy as np

if "/opt/trn_rl_repo" not in sys.path:
    sys.path.insert(0, "/opt/trn_rl_repo")

DEBUG = bool(os.environ.get("BASSDBG"))

B, S, D, H, DK = 2, 2048, 1024, 16, 64
HPC = 4            # heads per core
GW = HPC * DK      # 256
NCORES = 8
QC = 1024          # q-chunk width
NQC = S // QC      # 2
KT = 128           # k-tile
MASK_STEP = -240.0

_CACHE = {}


def _build_nc(reps=1):
    import concourse.bacc as bacc
    import concourse.tile as tile
    import concourse.bass as bass
    from concourse import mybir

    f32 = mybir.dt.float32
    bf = mybir.dt.bfloat16
    Exp = mybir.ActivationFunctionType.Exp
    PSUM = bass.MemorySpace.PSUM

    nc = bacc.Bacc(
        "TRN2",
        target_bir_lowering=False,
        debug=False,
        enable_asserts=False,
        num_devices=NCORES,
    )

    xT_d = nc.dram_tensor("xT", [D, S], bf, kind="ExternalInput")
    wq_d = nc.dram_tensor("wq", [D, GW], bf, kind="ExternalInput")
    wk_d = nc.dram_tensor("wk", [D, GW], bf, kind="ExternalInput")
    wv_d = nc.dram_tensor("wv", [D, GW], bf, kind="ExternalInput")
    wo_d = nc.dram_tensor("wo", [GW, D], bf, kind="ExternalInput")
    stA_d = nc.dram_tensor("stairA", [128, 128], bf, kind="ExternalInput")
    stB_d = nc.dram_tensor("stairB", [128, 128], bf, kind="ExternalInput")
    outT_d = nc.dram_tensor("outT", [D, S], f32, kind="ExternalOutput")
    scratch_d = nc.dram_tensor("nrm_scratch", [8, QC], f32)
    if DEBUG:
        dbg_sums_d = nc.dram_tensor("dbg_sums", [1, QC], f32, kind="ExternalOutput")
        dbg_rden_d = nc.dram_tensor("dbg_rden", [1, QC], f32, kind="ExternalOutput")
        dbg_bc_d = nc.dram_tensor("dbg_bc", [DK, QC], f32, kind="ExternalOutput")
        dbg_attnT_d = nc.dram_tensor("dbg_attnT", [128, 2, S], bf, kind="ExternalOutput")
        dbg_qT_d = nc.dram_tensor("dbg_qT", [128, 2, S], bf, kind="ExternalOutput")

    KC = D // 128  # 8 contraction chunks for the projections

    with tile.TileContext(nc) as tc:
        with (
            tc.tile_pool(name="weights", bufs=1) as wpool,
            tc.tile_pool(name="acts", bufs=1) as apool,
            tc.tile_pool(name="psmm", bufs=2, space=PSUM) as psmm,
            tc.tile_pool(name="psav", bufs=2, space=PSUM) as psav,
            tc.tile_pool(name="epool", bufs=8) as epool,
            tc.tile_pool(name="norm", bufs=3) as npool,
            tc.tile_pool(name="outp", bufs=4) as opool,
        ):
            # ---- loads ----
            # wq first, then the xT chunks: the first projection psum needs
            # wq plus all 8 xT chunks, so nothing else may delay them (the
            # stair constants are not needed until the first diagonal mask)
            stA = wpool.tile([128, 128], bf, tag="stA")
            stB = wpool.tile([128, 128], bf, tag="stB")
            wq_sb = wpool.tile([128, KC, GW], bf, tag="wq")
            wk_sb = wpool.tile([128, KC, GW], bf, tag="wk")
            wv_sb = wpool.tile([128, KC, GW], bf, tag="wv")
            wo_sb = wpool.tile([128, 2, D], bf, tag="wo")
            nc.sync.dma_start(wq_sb, wq_d.ap().rearrange("(kc p) m -> p kc m", p=128))

            first_rep = True
            for _rep in range(reps):  # >1 only for timing builds
                xT_sb = apool.tile([128, KC, S], bf, tag="xT", name=f"xT_sb{_rep}")
                xT_view = xT_d.ap().rearrange("(kc p) s -> p kc s", p=128)
                for kc in range(KC):
                    nc.sync.dma_start(xT_sb[:, kc, :], xT_view[:, kc, :])
                if first_rep:
                    first_rep = False
                    nc.sync.dma_start(
                        wk_sb, wk_d.ap().rearrange("(kc p) m -> p kc m", p=128))
                    nc.sync.dma_start(
                        wv_sb, wv_d.ap().rearrange("(kc p) m -> p kc m", p=128))
                    nc.sync.dma_start(
                        wo_sb, wo_d.ap().rearrange("(f p) n -> p f n", p=128))
                    nc.sync.dma_start(stA, stA_d.ap())
                    nc.sync.dma_start(stB, stB_d.ap())

                qT_sb = apool.tile([128, 2, S], bf, tag="qT")
                kT_sb = apool.tile([128, 2, S], bf, tag="kT")
                # head groups padded to 128 cols so AV matmul weights are
                # 128-wide (enables the compiler's fast-weight-load path);
                # cols [65,128) of each group are zeroed once on gpsimd
                v_sb = apool.tile([128, S // 128, HPC * 128], bf, tag="v")
                vpad = v_sb.rearrange("p st (h w) -> p st h w", w=128)
                nc.gpsimd.memset(vpad[:, :, :, DK + 1:128], 0.0)
                attnT = apool.tile([128, 2, S], bf, tag="attnT")

                def segs(vs):  # split [vs, QC) at the 512 psum-bank boundary
                    return [(vs, 512), (512, QC)] if vs < 512 else [(vs, QC)]

                def proj_qk(m, c2):
                    for name, w_sb, dst in (("q", wq_sb, qT_sb), ("k", wk_sb, kT_sb)):
                        ps = psmm.tile([128, QC], f32, tag="mm")
                        for kc in range(KC):
                            for a, b in segs(0):
                                nc.tensor.matmul(
                                    ps[:, a:b],
                                    lhsT=w_sb[:, kc, 128 * m:128 * (m + 1)],
                                    rhs=xT_sb[:, kc, QC * c2 + a:QC * c2 + b],
                                    start=(kc == 0),
                                    stop=(kc == KC - 1),
                                )
                        nc.vector.tensor_copy(dst[:, m, QC * c2:QC * (c2 + 1)], ps)

                def proj_v(st):
                    ps = psmm.tile([128, QC], f32, tag="mm")
                    for kc in range(KC):
                        nc.tensor.matmul(
                            ps[:, 0:GW],
                            lhsT=xT_sb[:, kc, 128 * st:128 * (st + 1)],
                            rhs=wv_sb[:, kc, :],
                            start=(kc == 0),
                            stop=(kc == KC - 1),
                        )
                    vdst = v_sb[:, st, :].rearrange("p (h w) -> p h w", w=128)
                    nc.vector.tensor_copy(
                        vdst[:, :, 0:DK],
                        ps[:, 0:GW].rearrange("p (h w) -> p h w", w=DK),
                    )
                    nc.vector.memset(vdst[:, :, DK:DK + 1], 1.0)

                def attention(mi, c):
                    # both heads of pair mi, q-chunk c; scores row-tile on PE
                    q0 = QC * c
                    njt = (q0 + QC) // KT
                    avs = []
                    for hh in range(2):
                        av = psav.tile([128, QC], f32, tag="av", name=f"av{hh}")
                        avs.append(av)
                    for j in range(njt):
                        k0 = KT * j
                        vs = max(0, k0 - q0)
                        pss = []
                        for hh in range(2):  # packed pair: bases 0 and 64
                            pb = 64 * hh
                            ps = psmm.tile([128, QC], f32, tag="mm")
                            for a, b in segs(vs):
                                diag_here = (k0 >= q0) and (a == vs)
                                nc.tensor.matmul(
                                    ps[:, a:b],
                                    lhsT=kT_sb[pb:pb + DK, mi, k0:k0 + KT],
                                    rhs=qT_sb[pb:pb + DK, mi, q0 + a:q0 + b],
                                    start=True,
                                    stop=not diag_here,
                                )
                                if diag_here:  # staircase causal mask on diag block
                                    nc.tensor.matmul(
                                        ps[:, vs:vs + KT],
                                        lhsT=stA,
                                        rhs=stB,
                                        start=False,
                                        stop=True,
                                    )
                            pss.append(ps)
                        # psum groups are tracked per 2KB bank: the first matmul
                        # touching a bank carries start, the last carries stop,
                        # partial-width writes in between are fine.
                        jA_last = q0 // KT + 3  # last j with vs < 512
                        av_ranges = []
                        if vs < 512:
                            av_ranges.append((vs, 512, j == jA_last))
                        av_ranges.append((max(vs, 512), QC, j == njt - 1))
                        for hh in range(2):
                            h = 2 * mi + hh
                            e = epool.tile([128, QC], bf, tag="e")
                            nc.scalar.activation(
                                e[:, vs:QC], pss[hh][:, vs:QC], Exp, scale=0.125
                            )
                            for a, b, fin in av_ranges:
                                nc.tensor.matmul(
                                    avs[hh][:, a:b],
                                    lhsT=v_sb[:, j, h * 128:(h + 1) * 128],
                                    rhs=e[:, a:b],
                                    start=(j == 0),
                                    stop=fin,
                                )
                    for hh in range(2):
                        av = avs[hh]
                        uid = (mi * 2 + c) * 2 + hh
                        rden = npool.tile([DK + 1, QC], f32, tag="rden")
                        nc.vector.reciprocal(rden[DK:DK + 1, :], av[DK:DK + 1, :])
                        # broadcast across partitions: bounce through DRAM (DMA
                        # cannot read a step-0 partition dim from SBUF, and
                        # compute engines cannot cross partition lanes)
                        sc = scratch_d.ap()[uid:uid + 1, :]
                        nc.sync.dma_start(sc, rden[DK:DK + 1, :])
                        bc = npool.tile([DK, QC], f32, tag="bc")
                        nc.sync.dma_start(bc, sc.to_broadcast([DK, QC]))
                        if DEBUG and mi == 1 and c == 1 and hh == 1:
                            dbg_s = npool.tile([DK + 1, QC], f32, tag="dbgs")
                            nc.vector.tensor_copy(dbg_s[DK:DK + 1, :], av[DK:DK + 1, :])
                            nc.sync.dma_start(dbg_sums_d.ap(), dbg_s[DK:DK + 1, :])
                            nc.sync.dma_start(dbg_rden_d.ap(), rden[DK:DK + 1, :])
                            nc.sync.dma_start(dbg_bc_d.ap(), bc)
                        if hh == 0:
                            nc.vector.tensor_mul(
                                attnT[0:DK, mi, q0:q0 + QC], av[0:DK, :], bc
                            )
                        else:
                            tmp = npool.tile([DK, QC], bf, tag="tmp")
                            nc.vector.tensor_mul(tmp, av[0:DK, :], bc)
                            nc.sync.dma_start(attnT[64:64 + DK, mi, q0:q0 + QC], tmp)

                def wo_proj(c2):  # output projection for one 1024-wide s-chunk
                    for dm in range(D // 128):
                        po = psmm.tile([128, QC], f32, tag="mm")
                        for f in range(2):
                            for a, b in segs(0):
                                nc.tensor.matmul(
                                    po[:, a:b],
                                    lhsT=wo_sb[:, f, 128 * dm:128 * (dm + 1)],
                                    rhs=attnT[:, f, QC * c2 + a:QC * c2 + b],
                                    start=(f == 0),
                                    stop=(f == 1),
                                )
                        ob = opool.tile([128, QC], f32, tag="ob")
                        nc.vector.tensor_copy(ob, po)
                        nc.sync.dma_start(
                            outT_d.ap()[128 * dm:128 * (dm + 1), QC * c2:QC * (c2 + 1)],
                            ob,
                        )

                # emission order: minimal prefix before attention can start;
                # later projections and the first wo chunk sit between attention
                # units so the scheduler can fill PE idle while attention waits
                # on ScalarE exp
                proj_qk(0, 0)
                proj_qk(1, 0)
                for st in range(8):
                    proj_v(st)
                attention(0, 0)
                attention(1, 0)
                proj_qk(0, 1)
                proj_qk(1, 1)
                for st in range(8, 16):
                    proj_v(st)
                attention(0, 1)
                attention(1, 1)
                wo_proj(0)
                wo_proj(1)

                if DEBUG:
                    nc.sync.dma_start(dbg_attnT_d.ap(), attnT)
                    nc.sync.dma_start(dbg_qT_d.ap(), qT_sb)

    nc.compile()
    return nc


def _get_nc():
    if "nc" not in _CACHE:
        _CACHE["nc"] = _build_nc()
    return _CACHE["nc"]


def _stairs():
    import ml_dtypes

    t = np.arange(128)
    stA = (t[:, None] <= t[None, :]).astype(ml_dtypes.bfloat16)
    stB = np.where(t[:, None] > t[None, :], MASK_STEP, 0.0).astype(ml_dtypes.bfloat16)
    return stA, stB


def _make_in_maps(x, wq, wk, wv, wo):
    import ml_dtypes

    bf = ml_dtypes.bfloat16
    stA, stB = _stairs()
    x = np.asarray(x, np.float32)
    xTs = [np.ascontiguousarray(x[b].T).astype(bf) for b in range(B)]
    wqb = np.asarray(wq, np.float32).astype(bf)
    wkb = np.asarray(wk, np.float32).astype(bf)
    wvb = np.asarray(wv, np.float32).astype(bf)
    wob = np.asarray(wo, np.float32).astype(bf)
    in_maps = []
    for c in range(NCORES):
        b, g = divmod(c, HPC)
        cols = slice(g * GW, (g + 1) * GW)
        in_maps.append({
            "xT": xTs[b],
            "wq": np.ascontiguousarray(wqb[:, cols]),
            "wk": np.ascontiguousarray(wkb[:, cols]),
            "wv": np.ascontiguousarray(wvb[:, cols]),
            "wo": np.ascontiguousarray(wob[cols, :]),
            "stairA": stA,
            "stairB": stB,
        })
    return in_maps


def run(x, wq, wk, wv, wo, trace=False):
    from concourse.bass_utils import run_bass_kernel_spmd

    nc = _get_nc()
    in_maps = _make_in_maps(x, wq, wk, wv, wo)
    res = run_bass_kernel_spmd(nc, in_maps, list(range(NCORES)), trace=trace)
    acc = np.zeros((B, D, S), np.float64)
    for c in range(NCORES):
        acc[c // HPC] += res.results[c]["outT"]
    out = np.ascontiguousarray(acc.transpose(0, 2, 1).astype(np.float32))
    return out, res


def kernel(x, wq, wk, wv, wo):
    out, _ = run(x, wq, wk, wv, wo, trace=False)
    return out



# revision 11
# speedup vs baseline: 1.0793x; 1.0793x over previous
"""Causal multi-head self-attention on 8 Trainium2 NeuronCores.

Problem: x[2,2048,1024], 16 heads, dk=64, causal softmax, fp32 in/out.

Sharding (data + tensor parallel, per the hint): core c handles batch
b = c//4 and head group g = c%4 (4 heads = 256 feature cols). wq/wk/wv
are column-sharded, wo row-sharded; each core returns a [D, S] fp16
partial of out^T for its batch, and the host sums the 4 partials per
batch in fp64.

Per-core kernel, fp16 operands (matmul accumulation fp32 in PSUM):
  - host supplies x^T [D, S]; q^T/k^T [256, S] = w^T @ x^T on PE,
    v [S, 256] natural, all computed on-device in fp16.
  - attention runs per head pair mi and 512-wide q-chunk c. Both heads
    of a pair share one scores psum tile [128, 2, 512] (one 2KB bank
    per head), so a single ScalarE exp (scale=1/8 fused, no
    max-subtraction: |scores/8|<~3; masked entries underflow to 0)
    covers both heads: e[128, 2, 512] fp16.
  - causal masking: k-tiles strictly above the diagonal are skipped;
    the diagonal 128x128 block gets a staircase additive mask
    (-240*(k-q) for k>q) from one extra fp16 matmul.
  - v is stored per head in 128-wide groups with parity layout:
    even heads [v(cols 0:64) | ones@64 | 0], odd heads
    [0 | ones@63 | v(cols 64:128)]. The AV matmul then lands even-head
    attn on psum rows 0:64 (denominator row 64) and odd-head attn on
    rows 64:128 (denominator row 63) -- partition-aligned with the
    attnT layout the wo matmul wants, so no cross-partition moves.
  - softmax normalize: DVE reciprocal on the two den rows (63, 64),
    then one K=2 PE matmul against a constant selector broadcasts
    1/den_even to partitions 0:64 and 1/den_odd to 64:128 in one psum
    tile (rhs dtype float32r: full fp32 data, 1 cycle/row). Two DVE
    tensor_muls write normalized attnT [128, 2, S] fp16.
  - out^T partial [D, S] fp16 = wo.T @ attnT on PE in [128,512] psum
    chunks; the f=0 (heads 0,1) contraction half is emitted as soon as
    pair 0's attnT chunk is ready so wo overlaps attention.
  - emission order interleaves projection / wo chunks between
    attention units as PE filler (attention is ScalarE-exp paced); a
    dedicated 2-buf [128,512] psum pool serves proj/wo/bc so fillers
    never contend with the scores/av psum pools.
"""

import os
import sys

import numpy as np

if "/opt/trn_rl_repo" not in sys.path:
    sys.path.insert(0, "/opt/trn_rl_repo")

DEBUG = bool(os.environ.get("BASSDBG"))

B, S, D, H, DK = 2, 2048, 1024, 16, 64
HPC = 4            # heads per core
GW = HPC * DK      # 256
NCORES = 8
QC = 512           # q-chunk width (1 psum bank of fp32)
NQC = S // QC      # 4
KT = 128           # k-tile
MASK_STEP = -240.0

_CACHE = {}


def _build_nc(reps=1):
    import concourse.bacc as bacc
    import concourse.tile as tile
    import concourse.bass as bass
    from concourse import mybir

    f32 = mybir.dt.float32
    f32r = mybir.dt.float32r
    fp16 = mybir.dt.float16
    Exp = mybir.ActivationFunctionType.Exp
    PSUM = bass.MemorySpace.PSUM

    nc = bacc.Bacc(
        "TRN2",
        target_bir_lowering=False,
        debug=False,
        enable_asserts=False,
        num_devices=NCORES,
    )

    xT_d = nc.dram_tensor("xT", [D, S], fp16, kind="ExternalInput")
    wq_d = nc.dram_tensor("wq", [D, GW], fp16, kind="ExternalInput")
    wk_d = nc.dram_tensor("wk", [D, GW], fp16, kind="ExternalInput")
    wv_d = nc.dram_tensor("wv", [D, GW], fp16, kind="ExternalInput")
    wo_d = nc.dram_tensor("wo", [GW, D], fp16, kind="ExternalInput")
    stA_d = nc.dram_tensor("stairA", [128, 128], fp16, kind="ExternalInput")
    stB_d = nc.dram_tensor("stairB", [128, 128], fp16, kind="ExternalInput")
    sel_d = nc.dram_tensor("sel", [65, 128], fp16, kind="ExternalInput")
    outT_d = nc.dram_tensor("outT", [D, S], fp16, kind="ExternalOutput")

    KC = D // 128  # 8 contraction chunks for the projections

    with tile.TileContext(nc) as tc:
        with (
            tc.tile_pool(name="weights", bufs=1) as wpool,
            tc.tile_pool(name="acts", bufs=1) as apool,
            tc.tile_pool(name="psmm", bufs=2, space=PSUM) as psmm,
            tc.tile_pool(name="psav", bufs=2, space=PSUM) as psav,
            tc.tile_pool(name="pflt", bufs=2, space=PSUM) as pflt,
            tc.tile_pool(name="epool", bufs=8) as epool,
            tc.tile_pool(name="norm", bufs=3) as npool,
            tc.tile_pool(name="outp", bufs=4) as opool,
        ):
            # ---- loads ----
            stA = wpool.tile([128, 128], fp16, tag="stA")
            stB = wpool.tile([128, 128], fp16, tag="stB")
            sel_sb = wpool.tile([65, 128], fp16, tag="sel")
            wq_sb = wpool.tile([128, KC, GW], fp16, tag="wq")
            wk_sb = wpool.tile([128, KC, GW], fp16, tag="wk")
            wv_sb = wpool.tile([128, KC, GW], fp16, tag="wv")
            wo_sb = wpool.tile([128, 2, D], fp16, tag="wo")
            nc.sync.dma_start(wq_sb, wq_d.ap().rearrange("(kc p) m -> p kc m", p=128))
            nc.sync.dma_start(wk_sb, wk_d.ap().rearrange("(kc p) m -> p kc m", p=128))

            first_rep = True
            for _rep in range(reps):  # >1 only for timing builds
                xT_sb = apool.tile([128, KC, S], fp16, tag="xT", name=f"xT_sb{_rep}")
                xT_view = xT_d.ap().rearrange("(kc p) s -> p kc s", p=128)
                for kc in range(2):
                    nc.sync.dma_start(xT_sb[:, kc, :], xT_view[:, kc, :])
                if first_rep:
                    first_rep = False
                    nc.sync.dma_start(
                        wv_sb, wv_d.ap().rearrange("(kc p) m -> p kc m", p=128))
                    nc.sync.dma_start(stA, stA_d.ap())
                    nc.sync.dma_start(stB, stB_d.ap())
                    nc.sync.dma_start(sel_sb, sel_d.ap())
                for kc in range(2, KC):
                    nc.sync.dma_start(xT_sb[:, kc, :], xT_view[:, kc, :])
                if _rep == 0:
                    nc.sync.dma_start(
                        wo_sb, wo_d.ap().rearrange("(f p) n -> p f n", p=128))

                qT_sb = apool.tile([128, 2, S], fp16, tag="qT")
                kT_sb = apool.tile([128, 2, S], fp16, tag="kT")
                # v per head in 128-wide groups, parity layout (see module
                # docstring); den rows land at psum partitions 64 (even
                # head) and 63 (odd head)
                v_sb = apool.tile([128, S // 128, HPC * 128], fp16, tag="v")
                vpar = v_sb.rearrange("p st (h2 par w) -> p st h2 par w",
                                      par=2, w=128)
                # even head: v @ cols 0:64, ones col 64 -> den row 64
                # odd head: ones col 32 -> den row 32, v @ cols 64:128
                # (matmul operand bases must be 0/32/64)
                nc.gpsimd.memset(vpar[:, :, :, 0, DK + 1:128], 0.0)
                nc.gpsimd.memset(vpar[:, :, :, 1, 0:32], 0.0)
                nc.gpsimd.memset(vpar[:, :, :, 1, 33:DK], 0.0)
                nc.gpsimd.memset(vpar[:, :, :, 0, DK:DK + 1], 1.0)
                nc.gpsimd.memset(vpar[:, :, :, 1, 32:33], 1.0)
                attnT = apool.tile([128, 2, S], fp16, tag="attnT")

                def proj_qk(dst_i, m, c2):
                    # q (dst_i=0) or k (dst_i=1), pair m, S-chunk c2 (512)
                    w_sb, dst = ((wq_sb, qT_sb), (wk_sb, kT_sb))[dst_i]
                    ps = pflt.tile([128, QC], f32, tag="flt")
                    for kc in range(KC):
                        nc.tensor.matmul(
                            ps,
                            lhsT=w_sb[:, kc, 128 * m:128 * (m + 1)],
                            rhs=xT_sb[:, kc, QC * c2:QC * (c2 + 1)],
                            start=(kc == 0),
                            stop=(kc == KC - 1),
                        )
                    nc.vector.tensor_copy(dst[:, m, QC * c2:QC * (c2 + 1)], ps)

                def proj_v(st):
                    ps = pflt.tile([128, QC], f32, tag="flt")
                    for kc in range(KC):
                        nc.tensor.matmul(
                            ps[:, 0:GW],
                            lhsT=xT_sb[:, kc, 128 * st:128 * (st + 1)],
                            rhs=wv_sb[:, kc, :],
                            start=(kc == 0),
                            stop=(kc == KC - 1),
                        )
                    pv = ps[:, 0:GW].rearrange("p (h2 par w) -> p h2 par w",
                                               par=2, w=DK)
                    nc.vector.tensor_copy(vpar[:, st, :, 0, 0:DK], pv[:, :, 0, :])
                    nc.vector.tensor_copy(vpar[:, st, :, 1, DK:128], pv[:, :, 1, :])

                def wo_unit(c2, dm):
                    # out^T chunk [128 rows dm, 512 cols c2]
                    po = pflt.tile([128, QC], f32, tag="flt")
                    for f in range(2):
                        nc.tensor.matmul(
                            po,
                            lhsT=wo_sb[:, f, 128 * dm:128 * (dm + 1)],
                            rhs=attnT[:, f, QC * c2:QC * (c2 + 1)],
                            start=(f == 0),
                            stop=(f == 1),
                        )
                    ob = opool.tile([128, QC], fp16, tag="ob")
                    nc.vector.tensor_copy(ob, po)
                    nc.sync.dma_start(
                        outT_d.ap()[128 * dm:128 * (dm + 1),
                                    QC * c2:QC * (c2 + 1)],
                        ob,
                    )

                def attention(mi, c):
                    # both heads of pair mi, q-chunk c (512 wide)
                    q0 = QC * c
                    njt = (q0 + QC) // KT
                    av_e = psav.tile([128, QC], f32, tag="av", name="av_e")
                    av_o = psav.tile([128, QC], f32, tag="av", name="av_o")
                    avs = (av_e, av_o)
                    for j in range(njt):
                        k0 = KT * j
                        vs = max(0, k0 - q0)
                        diag = k0 >= q0
                        ps = psmm.tile([128, 2, QC], f32, tag="mm")
                        for hh in range(2):
                            pb = 64 * hh
                            nc.tensor.matmul(
                                ps[:, hh, vs:QC],
                                lhsT=kT_sb[pb:pb + DK, mi, k0:k0 + KT],
                                rhs=qT_sb[pb:pb + DK, mi, q0 + vs:q0 + QC],
                                start=True,
                                stop=not diag,
                            )
                            if diag:  # staircase causal mask on diag block
                                nc.tensor.matmul(
                                    ps[:, hh, vs:vs + KT],
                                    lhsT=stA,
                                    rhs=stB,
                                    start=False,
                                    stop=True,
                                )
                        e = epool.tile([128, 2, QC], fp16, tag="e")
                        nc.scalar.activation(
                            e[:, :, vs:QC], ps[:, :, vs:QC], Exp, scale=0.125
                        )
                        for hh in range(2):
                            h = 2 * mi + hh
                            nc.tensor.matmul(
                                avs[hh][:, vs:QC],
                                lhsT=v_sb[:, j, h * 128:(h + 1) * 128],
                                rhs=e[:, hh, vs:QC],
                                start=(j == 0),
                                stop=(j == njt - 1),
                            )
                    # 1/den in fp16 (|rel err| ~ 5e-4, irrelevant); PE
                    # broadcast needs both matmul operands same dtype
                    rden = npool.tile([DK + 1, QC], fp16, tag="rden")
                    with nc.allow_low_precision(reason="fp16 1/den, err ~5e-4"):
                        nc.vector.reciprocal(rden[DK:DK + 1, :], av_e[DK:DK + 1, :])
                        nc.vector.reciprocal(rden[32:33, :], av_o[32:33, :])
                    bc = pflt.tile([128, QC], f32, tag="flt")
                    nc.tensor.matmul(
                        bc[0:DK, :],
                        lhsT=sel_sb[DK:DK + 1, 0:DK],
                        rhs=rden[DK:DK + 1, :],
                        start=True,
                        stop=True,
                    )
                    nc.tensor.matmul(
                        bc[DK:128, :],
                        lhsT=sel_sb[32:33, 0:DK],
                        rhs=rden[32:33, :],
                        start=True,
                        stop=True,
                    )
                    # tensor_tensor cannot take two PSUM inputs; stage the
                    # broadcast in SBUF once, shared by both heads
                    bc_sb = npool.tile([128, QC], f32, tag="bcsb")
                    nc.vector.tensor_copy(bc_sb, bc)
                    nc.vector.tensor_mul(
                        attnT[0:DK, mi, q0:q0 + QC], av_e[0:DK, :],
                        bc_sb[0:DK, :]
                    )
                    nc.vector.tensor_mul(
                        attnT[DK:128, mi, q0:q0 + QC], av_o[DK:128, :],
                        bc_sb[DK:128, :]
                    )

                # ---- emission order ----
                # minimal prefix before attention(0,0); projections, wo
                # chunks and later-chunk prereqs are emitted between
                # attention units so the tile scheduler can fill PE idle
                # while attention waits on ScalarE exp.
                proj_qk(0, 0, 0)
                proj_qk(1, 0, 0)
                for st in range(4):
                    proj_v(st)
                proj_qk(0, 1, 0)
                proj_qk(1, 1, 0)

                attention(0, 0)
                proj_qk(0, 0, 1)
                proj_qk(1, 0, 1)
                attention(1, 0)
                for st in range(4, 8):
                    proj_v(st)
                proj_qk(0, 1, 1)
                proj_qk(1, 1, 1)

                attention(0, 1)
                for dm in range(4):
                    wo_unit(0, dm)
                proj_qk(0, 0, 2)
                proj_qk(1, 0, 2)
                attention(1, 1)
                for dm in range(4, 8):
                    wo_unit(0, dm)
                for st in range(8, 12):
                    proj_v(st)
                proj_qk(0, 1, 2)
                proj_qk(1, 1, 2)

                attention(0, 2)
                for dm in range(4):
                    wo_unit(1, dm)
                proj_qk(0, 0, 3)
                proj_qk(1, 0, 3)
                attention(1, 2)
                for dm in range(4, 8):
                    wo_unit(1, dm)
                for st in range(12, 16):
                    proj_v(st)
                proj_qk(0, 1, 3)
                proj_qk(1, 1, 3)

                attention(0, 3)
                for dm in range(4):
                    wo_unit(2, dm)
                attention(1, 3)
                for dm in range(4, 8):
                    wo_unit(2, dm)
                for dm in range(8):
                    wo_unit(3, dm)

    nc.compile()
    return nc


def _get_nc():
    if "nc" not in _CACHE:
        _CACHE["nc"] = _build_nc()
    return _CACHE["nc"]


def _stairs():
    t = np.arange(128)
    stA = (t[:, None] <= t[None, :]).astype(np.float16)
    stB = np.where(t[:, None] > t[None, :], MASK_STEP, 0.0).astype(np.float16)
    return stA, stB


def _sel():
    # ones rows for the 1/den broadcast matmuls: row 64 (even-head den)
    # and row 32 (odd-head den); lhsT free size 64 -> 64 out partitions
    sel = np.zeros((65, 128), np.float16)
    sel[64, 0:64] = 1.0
    sel[32, 0:64] = 1.0
    return sel


def _make_in_maps(x, wq, wk, wv, wo):
    stA, stB = _stairs()
    sel = _sel()
    x = np.asarray(x, np.float32)
    xTs = [np.ascontiguousarray(x[b].T).astype(np.float16) for b in range(B)]
    wqh = np.asarray(wq, np.float32).astype(np.float16)
    wkh = np.asarray(wk, np.float32).astype(np.float16)
    wvh = np.asarray(wv, np.float32).astype(np.float16)
    woh = np.asarray(wo, np.float32).astype(np.float16)
    in_maps = []
    for c in range(NCORES):
        b, g = divmod(c, HPC)
        cols = slice(g * GW, (g + 1) * GW)
        in_maps.append({
            "xT": xTs[b],
            "wq": np.ascontiguousarray(wqh[:, cols]),
            "wk": np.ascontiguousarray(wkh[:, cols]),
            "wv": np.ascontiguousarray(wvh[:, cols]),
            "wo": np.ascontiguousarray(woh[cols, :]),
            "stairA": stA,
            "stairB": stB,
            "sel": sel,
        })
    return in_maps


def run(x, wq, wk, wv, wo, trace=False):
    from concourse.bass_utils import run_bass_kernel_spmd

    nc = _get_nc()
    in_maps = _make_in_maps(x, wq, wk, wv, wo)
    res = run_bass_kernel_spmd(nc, in_maps, list(range(NCORES)), trace=trace)
    acc = np.zeros((B, D, S), np.float64)
    for c in range(NCORES):
        acc[c // HPC] += res.results[c]["outT"]
    out = np.ascontiguousarray(acc.transpose(0, 2, 1).astype(np.float32))
    return out, res


def kernel(x, wq, wk, wv, wo):
    out, _ = run(x, wq, wk, wv, wo, trace=False)
    return out


# revision 18
# speedup vs baseline: 1.0958x; 1.0154x over previous
"""Causal multi-head self-attention on 8 Trainium2 NeuronCores.

Problem: x[2,2048,1024], 16 heads, dk=64, causal softmax, fp32 in/out.

Sharding (data + tensor parallel, per the hint): core c handles batch
b = c//4 and head group g = c%4 (4 heads = 256 feature cols). wq/wk/wv
are column-sharded, wo row-sharded; each core returns a [D, S] fp16
partial of out^T for its batch, and the host sums the 4 partials per
batch in fp64.

Per-core kernel, fp16 operands (matmul accumulation fp32 in PSUM):
  - host supplies x^T [D, S]; q^T/k^T [256, S] = w^T @ x^T on PE,
    v [S, 256] natural, all computed on-device in fp16.
  - attention runs per head pair mi and 512-wide q-chunk c. Both heads
    of a pair share one scores psum tile [128, 2, 512] (one 2KB bank
    per head), so a single ScalarE exp (scale=1/8 fused, no
    max-subtraction: |scores/8|<~3; masked entries underflow to 0)
    covers both heads: e[128, 2, 512] fp16.
  - causal masking: k-tiles strictly above the diagonal are skipped;
    the diagonal 128x128 block gets a staircase additive mask
    (-240*(k-q) for k>q) from one extra fp16 matmul.
  - v is stored per head in 128-wide groups with parity layout:
    even heads [v(cols 0:64) | ones@64 | 0], odd heads
    [0 | ones@63 | v(cols 64:128)]. The AV matmul then lands even-head
    attn on psum rows 0:64 (denominator row 64) and odd-head attn on
    rows 64:128 (denominator row 63) -- partition-aligned with the
    attnT layout the wo matmul wants, so no cross-partition moves.
  - softmax normalize: DVE reciprocal on the two den rows (63, 64),
    then one K=2 PE matmul against a constant selector broadcasts
    1/den_even to partitions 0:64 and 1/den_odd to 64:128 in one psum
    tile (rhs dtype float32r: full fp32 data, 1 cycle/row). Two DVE
    tensor_muls write normalized attnT [128, 2, S] fp16.
  - out^T partial [D, S] fp16 = wo.T @ attnT on PE in [128,512] psum
    chunks; the f=0 (heads 0,1) contraction half is emitted as soon as
    pair 0's attnT chunk is ready so wo overlaps attention.
  - emission order interleaves projection / wo chunks between
    attention units as PE filler (attention is ScalarE-exp paced); a
    dedicated 2-buf [128,512] psum pool serves proj/wo/bc so fillers
    never contend with the scores/av psum pools.
"""

import os
import sys

import numpy as np

if "/opt/trn_rl_repo" not in sys.path:
    sys.path.insert(0, "/opt/trn_rl_repo")

DEBUG = bool(os.environ.get("BASSDBG"))

B, S, D, H, DK = 2, 2048, 1024, 16, 64
HPC = 4            # heads per core
GW = HPC * DK      # 256
NCORES = 8
QC = 512           # q-chunk width (1 psum bank of fp32)
NQC = S // QC      # 4
KT = 128           # k-tile
MASK_STEP = -240.0

_CACHE = {}


def _build_nc(reps=1):
    import concourse.bacc as bacc
    import concourse.tile as tile
    import concourse.bass as bass
    from concourse import mybir

    f32 = mybir.dt.float32
    f32r = mybir.dt.float32r
    fp16 = mybir.dt.float16
    Exp = mybir.ActivationFunctionType.Exp
    PSUM = bass.MemorySpace.PSUM

    nc = bacc.Bacc(
        "TRN2",
        target_bir_lowering=False,
        debug=False,
        enable_asserts=False,
        num_devices=NCORES,
    )

    xT_d = nc.dram_tensor("xT", [D, S], fp16, kind="ExternalInput")
    wq_d = nc.dram_tensor("wq", [D, GW], fp16, kind="ExternalInput")
    wk_d = nc.dram_tensor("wk", [D, GW], fp16, kind="ExternalInput")
    wv_d = nc.dram_tensor("wv", [D, GW], fp16, kind="ExternalInput")
    wo_d = nc.dram_tensor("wo", [GW, D], fp16, kind="ExternalInput")
    stA_d = nc.dram_tensor("stairA", [128, 128], fp16, kind="ExternalInput")
    stB_d = nc.dram_tensor("stairB", [128, 128], fp16, kind="ExternalInput")
    sel_d = nc.dram_tensor("sel", [65, 128], fp16, kind="ExternalInput")
    outT_d = nc.dram_tensor("outT", [D, S], fp16, kind="ExternalOutput")

    KC = D // 128  # 8 contraction chunks for the projections

    with tile.TileContext(nc) as tc:
        with (
            tc.tile_pool(name="weights", bufs=1) as wpool,
            tc.tile_pool(name="acts", bufs=1) as apool,
            tc.tile_pool(name="psmm", bufs=2, space=PSUM) as psmm,
            tc.tile_pool(name="psav", bufs=3, space=PSUM) as psav,
            tc.tile_pool(name="pflt", bufs=1, space=PSUM) as pflt,
            tc.tile_pool(name="epool", bufs=8) as epool,
            tc.tile_pool(name="norm", bufs=3) as npool,
            tc.tile_pool(name="outp", bufs=4) as opool,
        ):
            # ---- loads ----
            stA = wpool.tile([128, 128], fp16, tag="stA")
            stB = wpool.tile([128, 128], fp16, tag="stB")
            sel_sb = wpool.tile([65, 128], fp16, tag="sel")
            wq_sb = wpool.tile([128, KC, GW], fp16, tag="wq")
            wk_sb = wpool.tile([128, KC, GW], fp16, tag="wk")
            wv_sb = wpool.tile([128, KC, GW], fp16, tag="wv")
            wo_sb = wpool.tile([128, 2, D], fp16, tag="wo")
            nc.sync.dma_start(wq_sb, wq_d.ap().rearrange("(kc p) m -> p kc m", p=128))
            nc.sync.dma_start(wk_sb, wk_d.ap().rearrange("(kc p) m -> p kc m", p=128))

            first_rep = True
            for _rep in range(reps):  # >1 only for timing builds
                xT_sb = apool.tile([128, KC, S], fp16, tag="xT", name=f"xT_sb{_rep}")
                xT_view = xT_d.ap().rearrange("(kc p) s -> p kc s", p=128)
                for kc in range(2):
                    nc.sync.dma_start(xT_sb[:, kc, :], xT_view[:, kc, :])
                if first_rep:
                    first_rep = False
                    nc.sync.dma_start(
                        wv_sb, wv_d.ap().rearrange("(kc p) m -> p kc m", p=128))
                    nc.sync.dma_start(stA, stA_d.ap())
                    nc.sync.dma_start(stB, stB_d.ap())
                    nc.sync.dma_start(sel_sb, sel_d.ap())
                for kc in range(2, KC):
                    nc.sync.dma_start(xT_sb[:, kc, :], xT_view[:, kc, :])
                if _rep == 0:
                    nc.sync.dma_start(
                        wo_sb, wo_d.ap().rearrange("(f p) n -> p f n", p=128))

                qT_sb = apool.tile([128, 2, S], fp16, tag="qT")
                kT_sb = apool.tile([128, 2, S], fp16, tag="kT")
                # v per head in 128-wide groups, parity layout (see module
                # docstring); den rows land at psum partitions 64 (even
                # head) and 63 (odd head)
                v_sb = apool.tile([128, S // 128, HPC * 128], fp16, tag="v")
                vpar = v_sb.rearrange("p st (h2 par w) -> p st h2 par w",
                                      par=2, w=128)
                # even head: v @ cols 0:64, ones col 64 -> den row 64
                # odd head: ones col 32 -> den row 32, v @ cols 64:128
                # (matmul operand bases must be 0/32/64)
                nc.gpsimd.memset(vpar[:, :, :, 0, DK + 1:128], 0.0)
                nc.gpsimd.memset(vpar[:, :, :, 1, 0:32], 0.0)
                nc.gpsimd.memset(vpar[:, :, :, 1, 33:DK], 0.0)
                nc.gpsimd.memset(vpar[:, :, :, 0, DK:DK + 1], 1.0)
                nc.gpsimd.memset(vpar[:, :, :, 1, 32:33], 1.0)
                attnT = apool.tile([128, 2, S], fp16, tag="attnT")

                def proj_qk(dst_i, m, c2):
                    # q (dst_i=0) or k (dst_i=1), pair m, S-chunk c2 (512)
                    w_sb, dst = ((wq_sb, qT_sb), (wk_sb, kT_sb))[dst_i]
                    ps = pflt.tile([128, QC], f32, tag="flt")
                    for kc in range(KC):
                        nc.tensor.matmul(
                            ps,
                            lhsT=w_sb[:, kc, 128 * m:128 * (m + 1)],
                            rhs=xT_sb[:, kc, QC * c2:QC * (c2 + 1)],
                            start=(kc == 0),
                            stop=(kc == KC - 1),
                        )
                    nc.vector.tensor_copy(dst[:, m, QC * c2:QC * (c2 + 1)], ps)

                def proj_qk2(m):
                    # startup variant: q and k for chunk 0 share one psmm
                    # tile (2 banks), interleaved per xT chunk arrival so PE
                    # is never gated on the 1-buf filler pool during load
                    ps = psmm.tile([128, 2, QC], f32, tag="mm")
                    for kc in range(KC):
                        for di, w_sb in ((0, wq_sb), (1, wk_sb)):
                            nc.tensor.matmul(
                                ps[:, di, :],
                                lhsT=w_sb[:, kc, 128 * m:128 * (m + 1)],
                                rhs=xT_sb[:, kc, 0:QC],
                                start=(kc == 0),
                                stop=(kc == KC - 1),
                            )
                    nc.vector.tensor_copy(qT_sb[:, m, 0:QC], ps[:, 0, :])
                    nc.vector.tensor_copy(kT_sb[:, m, 0:QC], ps[:, 1, :])

                def proj_v(st, pool=None):
                    ps = (pool or pflt).tile([128, QC], f32,
                                             tag="av" if pool else "flt")
                    for kc in range(KC):
                        nc.tensor.matmul(
                            ps[:, 0:GW],
                            lhsT=xT_sb[:, kc, 128 * st:128 * (st + 1)],
                            rhs=wv_sb[:, kc, :],
                            start=(kc == 0),
                            stop=(kc == KC - 1),
                        )
                    pv = ps[:, 0:GW].rearrange("p (h2 par w) -> p h2 par w",
                                               par=2, w=DK)
                    nc.vector.tensor_copy(vpar[:, st, :, 0, 0:DK], pv[:, :, 0, :])
                    nc.vector.tensor_copy(vpar[:, st, :, 1, DK:128], pv[:, :, 1, :])

                def wo_unit(c2, dm, po=None):
                    # out^T chunk [128 rows dm, 512 cols c2]
                    if po is None:
                        po = pflt.tile([128, QC], f32, tag="flt")
                    for f in range(2):
                        nc.tensor.matmul(
                            po,
                            lhsT=wo_sb[:, f, 128 * dm:128 * (dm + 1)],
                            rhs=attnT[:, f, QC * c2:QC * (c2 + 1)],
                            start=(f == 0),
                            stop=(f == 1),
                        )
                    ob = opool.tile([128, QC], fp16, tag="ob")
                    nc.vector.tensor_copy(ob, po)
                    nc.sync.dma_start(
                        outT_d.ap()[128 * dm:128 * (dm + 1),
                                    QC * c2:QC * (c2 + 1)],
                        ob,
                    )

                def attention(mi, c):
                    # both heads of pair mi, q-chunk c (512 wide)
                    q0 = QC * c
                    njt = (q0 + QC) // KT
                    av_e = psav.tile([128, QC], f32, tag="av", name="av_e")
                    av_o = psav.tile([128, QC], f32, tag="av", name="av_o")
                    avs = (av_e, av_o)
                    for j in range(njt):
                        k0 = KT * j
                        vs = max(0, k0 - q0)
                        diag = k0 >= q0
                        ps = psmm.tile([128, 2, QC], f32, tag="mm")
                        for hh in range(2):
                            pb = 64 * hh
                            nc.tensor.matmul(
                                ps[:, hh, vs:QC],
                                lhsT=kT_sb[pb:pb + DK, mi, k0:k0 + KT],
                                rhs=qT_sb[pb:pb + DK, mi, q0 + vs:q0 + QC],
                                start=True,
                                stop=not diag,
                            )
                            if diag:  # staircase causal mask on diag block
                                nc.tensor.matmul(
                                    ps[:, hh, vs:vs + KT],
                                    lhsT=stA,
                                    rhs=stB,
                                    start=False,
                                    stop=True,
                                )
                        e = epool.tile([128, 2, QC], fp16, tag="e")
                        nc.scalar.activation(
                            e[:, :, vs:QC], ps[:, :, vs:QC], Exp, scale=0.125
                        )
                        for hh in range(2):
                            h = 2 * mi + hh
                            nc.tensor.matmul(
                                avs[hh][:, vs:QC],
                                lhsT=v_sb[:, j, h * 128:(h + 1) * 128],
                                rhs=e[:, hh, vs:QC],
                                start=(j == 0),
                                stop=(j == njt - 1),
                            )
                    # 1/den in fp16 (|rel err| ~ 5e-4, irrelevant); PE
                    # broadcast needs both matmul operands same dtype
                    rden = npool.tile([DK + 1, QC], fp16, tag="rden")
                    with nc.allow_low_precision(reason="fp16 1/den, err ~5e-4"):
                        nc.vector.reciprocal(rden[DK:DK + 1, :], av_e[DK:DK + 1, :])
                        nc.vector.reciprocal(rden[32:33, :], av_o[32:33, :])
                    bcps = psmm.tile([128, 2, QC], f32, tag="mm")
                    bc = bcps[:, 0, :]
                    nc.tensor.matmul(
                        bc[0:DK, :],
                        lhsT=sel_sb[DK:DK + 1, 0:DK],
                        rhs=rden[DK:DK + 1, :],
                        start=True,
                        stop=True,
                    )
                    nc.tensor.matmul(
                        bc[DK:128, :],
                        lhsT=sel_sb[32:33, 0:DK],
                        rhs=rden[32:33, :],
                        start=True,
                        stop=True,
                    )
                    # tensor_tensor cannot take two PSUM inputs; stage the
                    # broadcast in SBUF once, shared by both heads
                    bc_sb = npool.tile([128, QC], f32, tag="bcsb")
                    nc.vector.tensor_copy(bc_sb, bc)
                    nc.vector.tensor_mul(
                        attnT[0:DK, mi, q0:q0 + QC], av_e[0:DK, :],
                        bc_sb[0:DK, :]
                    )
                    nc.vector.tensor_mul(
                        attnT[DK:128, mi, q0:q0 + QC], av_o[DK:128, :],
                        bc_sb[DK:128, :]
                    )

                # ---- emission order ----
                # minimal prefix before attention(0,0); projections, wo
                # chunks and later-chunk prereqs are emitted between
                # attention units so the tile scheduler can fill PE idle
                # while attention waits on ScalarE exp.
                #
                # The prefix is DMA-gated (xT chunks arrive over ~16us), so
                # every prefix unit borrows an idle psum bank: 2 psmm tiles
                # (q+k pairs), 3 psav bufs (v0-2), 1 pflt (v3) -- all 8
                # banks hold a concurrent prefix unit.
                proj_qk2(0)
                proj_qk2(1)
                for st in range(3):
                    proj_v(st, pool=psav)
                proj_v(3)

                attention(0, 0)
                proj_qk(0, 0, 1)
                proj_qk(1, 0, 1)
                attention(1, 0)
                for st in range(4, 8):
                    proj_v(st)
                proj_qk(0, 1, 1)
                proj_qk(1, 1, 1)

                attention(0, 1)
                for dm in range(4):
                    wo_unit(0, dm)
                proj_qk(0, 0, 2)
                proj_qk(1, 0, 2)
                attention(1, 1)
                for dm in range(4, 8):
                    wo_unit(0, dm)
                for st in range(8, 12):
                    proj_v(st)
                proj_qk(0, 1, 2)
                proj_qk(1, 1, 2)

                attention(0, 2)
                for dm in range(4):
                    wo_unit(1, dm)
                proj_qk(0, 0, 3)
                proj_qk(1, 0, 3)
                attention(1, 2)
                for dm in range(4, 8):
                    wo_unit(1, dm)
                for st in range(12, 16):
                    proj_v(st)
                proj_qk(0, 1, 3)
                proj_qk(1, 1, 3)

                attention(0, 3)
                for dm in range(4):
                    wo_unit(2, dm)
                attention(1, 3)
                for dm in range(4, 8):
                    wo_unit(2, dm)
                # tail: attention is done, scores pool is free -- run the
                # last wo chunk two-wide through psmm half-tiles
                for dm2 in range(4):
                    po2 = psmm.tile([128, 2, QC], f32, tag="mm")
                    wo_unit(3, 2 * dm2, po=po2[:, 0, :])
                    wo_unit(3, 2 * dm2 + 1, po=po2[:, 1, :])

    nc.compile()
    return nc


def _get_nc():
    if "nc" not in _CACHE:
        _CACHE["nc"] = _build_nc()
    return _CACHE["nc"]


def _stairs():
    t = np.arange(128)
    stA = (t[:, None] <= t[None, :]).astype(np.float16)
    stB = np.where(t[:, None] > t[None, :], MASK_STEP, 0.0).astype(np.float16)
    return stA, stB


def _sel():
    # ones rows for the 1/den broadcast matmuls: row 64 (even-head den)
    # and row 32 (odd-head den); lhsT free size 64 -> 64 out partitions
    sel = np.zeros((65, 128), np.float16)
    sel[64, 0:64] = 1.0
    sel[32, 0:64] = 1.0
    return sel


def _make_in_maps(x, wq, wk, wv, wo):
    stA, stB = _stairs()
    sel = _sel()
    x = np.asarray(x, np.float32)
    xTs = [np.ascontiguousarray(x[b].T).astype(np.float16) for b in range(B)]
    wqh = np.asarray(wq, np.float32).astype(np.float16)
    wkh = np.asarray(wk, np.float32).astype(np.float16)
    wvh = np.asarray(wv, np.float32).astype(np.float16)
    woh = np.asarray(wo, np.float32).astype(np.float16)
    in_maps = []
    for c in range(NCORES):
        b, g = divmod(c, HPC)
        cols = slice(g * GW, (g + 1) * GW)
        in_maps.append({
            "xT": xTs[b],
            "wq": np.ascontiguousarray(wqh[:, cols]),
            "wk": np.ascontiguousarray(wkh[:, cols]),
            "wv": np.ascontiguousarray(wvh[:, cols]),
            "wo": np.ascontiguousarray(woh[cols, :]),
            "stairA": stA,
            "stairB": stB,
            "sel": sel,
        })
    return in_maps


def run(x, wq, wk, wv, wo, trace=False):
    from concourse.bass_utils import run_bass_kernel_spmd

    nc = _get_nc()
    in_maps = _make_in_maps(x, wq, wk, wv, wo)
    res = run_bass_kernel_spmd(nc, in_maps, list(range(NCORES)), trace=trace)
    acc = np.zeros((B, D, S), np.float64)
    for c in range(NCORES):
        acc[c // HPC] += res.results[c]["outT"]
    out = np.ascontiguousarray(acc.transpose(0, 2, 1).astype(np.float32))
    return out, res


def kernel(x, wq, wk, wv, wo):
    out, _ = run(x, wq, wk, wv, wo, trace=False)
    return out


# revision 24
# speedup vs baseline: 1.1140x; 1.0166x over previous
"""Causal multi-head self-attention on 8 Trainium2 NeuronCores.

Problem: x[2,2048,1024], 16 heads, dk=64, causal softmax, fp32 in/out.

Sharding (data + tensor parallel, per the hint): core c handles batch
b = c//4 and head group g = c%4 (4 heads = 256 feature cols). wq/wk/wv
are column-sharded, wo row-sharded; each core returns a [D, S] fp16
partial of out^T for its batch, and the host sums the 4 partials per
batch in fp64.

Per-core kernel, fp16 operands (matmul accumulation fp32 in PSUM):
  - host supplies x^T [D, S]; q^T/k^T [256, S] = w^T @ x^T on PE,
    v [S, 256] natural, all computed on-device in fp16.
  - attention runs per head pair mi and 512-wide q-chunk c. Both heads
    of a pair share one scores psum tile [128, 2, 512] (one 2KB bank
    per head), so a single ScalarE exp (scale=1/8 fused, no
    max-subtraction: |scores/8|<~3; masked entries underflow to 0)
    covers both heads: e[128, 2, 512] fp16.
  - causal masking: k-tiles strictly above the diagonal are skipped;
    the diagonal 128x128 block gets a staircase additive mask
    (-240*(k-q) for k>q) from one extra fp16 matmul.
  - v is stored per head in 128-wide groups with parity layout:
    even heads [v(cols 0:64) | ones@64 | 0], odd heads
    [0 | ones@63 | v(cols 64:128)]. The AV matmul then lands even-head
    attn on psum rows 0:64 (denominator row 64) and odd-head attn on
    rows 64:128 (denominator row 63) -- partition-aligned with the
    attnT layout the wo matmul wants, so no cross-partition moves.
  - softmax normalize: DVE reciprocal on the two den rows (63, 64),
    then one K=2 PE matmul against a constant selector broadcasts
    1/den_even to partitions 0:64 and 1/den_odd to 64:128 in one psum
    tile (rhs dtype float32r: full fp32 data, 1 cycle/row). Two DVE
    tensor_muls write normalized attnT [128, 2, S] fp16.
  - out^T partial [D, S] fp16 = wo.T @ attnT on PE in [128,512] psum
    chunks; the f=0 (heads 0,1) contraction half is emitted as soon as
    pair 0's attnT chunk is ready so wo overlaps attention.
  - emission order interleaves projection / wo chunks between
    attention units as PE filler (attention is ScalarE-exp paced); a
    dedicated 2-buf [128,512] psum pool serves proj/wo/bc so fillers
    never contend with the scores/av psum pools.
"""

import os
import sys

import numpy as np

if "/opt/trn_rl_repo" not in sys.path:
    sys.path.insert(0, "/opt/trn_rl_repo")

DEBUG = bool(os.environ.get("BASSDBG"))

B, S, D, H, DK = 2, 2048, 1024, 16, 64
HPC = 4            # heads per core
GW = HPC * DK      # 256
NCORES = 8
QC = 512           # q-chunk width (1 psum bank of fp32)
NQC = S // QC      # 4
KT = 128           # k-tile
MASK_STEP = -240.0

_CACHE = {}


def _build_nc(reps=1):
    import concourse.bacc as bacc
    import concourse.tile as tile
    import concourse.bass as bass
    from concourse import mybir

    f32 = mybir.dt.float32
    f32r = mybir.dt.float32r
    fp16 = mybir.dt.float16
    Exp = mybir.ActivationFunctionType.Exp
    PSUM = bass.MemorySpace.PSUM

    nc = bacc.Bacc(
        "TRN2",
        target_bir_lowering=False,
        debug=False,
        enable_asserts=False,
        num_devices=NCORES,
    )

    xT_d = nc.dram_tensor("xT", [D, S], fp16, kind="ExternalInput")
    wq_d = nc.dram_tensor("wq", [D, GW], fp16, kind="ExternalInput")
    wk_d = nc.dram_tensor("wk", [D, GW], fp16, kind="ExternalInput")
    wv_d = nc.dram_tensor("wv", [D, GW], fp16, kind="ExternalInput")
    wo_d = nc.dram_tensor("wo", [GW, D], fp16, kind="ExternalInput")
    stA_d = nc.dram_tensor("stairA", [128, 128], fp16, kind="ExternalInput")
    stB_d = nc.dram_tensor("stairB", [128, 128], fp16, kind="ExternalInput")
    sel_d = nc.dram_tensor("sel", [65, 128], fp16, kind="ExternalInput")
    outT_d = nc.dram_tensor("outT", [D, S], fp16, kind="ExternalOutput")

    KC = D // 128  # 8 contraction chunks for the projections

    with tile.TileContext(nc) as tc:
        with (
            tc.tile_pool(name="weights", bufs=1) as wpool,
            tc.tile_pool(name="acts", bufs=1) as apool,
            tc.tile_pool(name="psmm", bufs=2, space=PSUM) as psmm,
            tc.tile_pool(name="psav", bufs=3, space=PSUM) as psav,
            tc.tile_pool(name="pflt", bufs=1, space=PSUM) as pflt,
            tc.tile_pool(name="epool", bufs=8) as epool,
            tc.tile_pool(name="norm", bufs=3) as npool,
            tc.tile_pool(name="outp", bufs=4) as opool,
        ):
            # ---- loads ----
            stA = wpool.tile([128, 128], fp16, tag="stA")
            stB = wpool.tile([128, 128], fp16, tag="stB")
            sel_sb = wpool.tile([65, 128], fp16, tag="sel")
            wq_sb = wpool.tile([128, KC, GW], fp16, tag="wq")
            wk_sb = wpool.tile([128, KC, GW], fp16, tag="wk")
            wv_sb = wpool.tile([128, KC, GW], fp16, tag="wv")
            wo_sb = wpool.tile([128, 2, D], fp16, tag="wo")
            nc.sync.dma_start(wq_sb, wq_d.ap().rearrange("(kc p) m -> p kc m", p=128))

            first_rep = True
            for _rep in range(reps):  # >1 only for timing builds
                xT_sb = apool.tile([128, KC, S], fp16, tag="xT", name=f"xT_sb{_rep}")
                xT_view = xT_d.ap().rearrange("(kc p) s -> p kc s", p=128)
                nc.sync.dma_start(xT_sb[:, 0, :], xT_view[:, 0, :])
                if first_rep:
                    first_rep = False
                    nc.sync.dma_start(
                        wk_sb, wk_d.ap().rearrange("(kc p) m -> p kc m", p=128))
                    nc.sync.dma_start(
                        wv_sb, wv_d.ap().rearrange("(kc p) m -> p kc m", p=128))
                    nc.sync.dma_start(stA, stA_d.ap())
                    nc.sync.dma_start(stB, stB_d.ap())
                    nc.sync.dma_start(sel_sb, sel_d.ap())
                for kc in range(1, KC):
                    nc.sync.dma_start(xT_sb[:, kc, :], xT_view[:, kc, :])
                if _rep == 0:
                    nc.sync.dma_start(
                        wo_sb, wo_d.ap().rearrange("(f p) n -> p f n", p=128))

                qT_sb = apool.tile([128, 2, S], fp16, tag="qT")
                kT_sb = apool.tile([128, 2, S], fp16, tag="kT")
                # v per head in 128-wide groups, parity layout (see module
                # docstring); den rows land at psum partitions 64 (even
                # head) and 63 (odd head)
                v_sb = apool.tile([128, S // 128, HPC * 128], fp16, tag="v")
                vpar = v_sb.rearrange("p st (h2 par w) -> p st h2 par w",
                                      par=2, w=128)
                # even head: v @ cols 0:64, ones col 64 -> den row 64
                # odd head: ones col 32 -> den row 32, v @ cols 64:128
                # (matmul operand bases must be 0/32/64)
                nc.gpsimd.memset(vpar[:, :, :, 0, DK + 1:128], 0.0)
                nc.gpsimd.memset(vpar[:, :, :, 1, 0:32], 0.0)
                nc.gpsimd.memset(vpar[:, :, :, 1, 33:DK], 0.0)
                nc.gpsimd.memset(vpar[:, :, :, 0, DK:DK + 1], 1.0)
                nc.gpsimd.memset(vpar[:, :, :, 1, 32:33], 1.0)
                attnT = apool.tile([128, 2, S], fp16, tag="attnT")

                def proj_qk(dst_i, m, c2):
                    # q (dst_i=0) or k (dst_i=1), pair m, S-chunk c2 (512)
                    w_sb, dst = ((wq_sb, qT_sb), (wk_sb, kT_sb))[dst_i]
                    ps = pflt.tile([128, QC], f32, tag="flt")
                    for kc in range(KC):
                        nc.tensor.matmul(
                            ps,
                            lhsT=w_sb[:, kc, 128 * m:128 * (m + 1)],
                            rhs=xT_sb[:, kc, QC * c2:QC * (c2 + 1)],
                            start=(kc == 0),
                            stop=(kc == KC - 1),
                        )
                    nc.vector.tensor_copy(dst[:, m, QC * c2:QC * (c2 + 1)], ps)

                def proj_qk2(m):
                    # startup variant: q and k for chunk 0 share one psmm
                    # tile (2 banks), interleaved per xT chunk arrival so PE
                    # is never gated on the 1-buf filler pool during load
                    ps = psmm.tile([128, 2, QC], f32, tag="mm")
                    for kc in range(KC):
                        for di, w_sb in ((0, wq_sb), (1, wk_sb)):
                            nc.tensor.matmul(
                                ps[:, di, :],
                                lhsT=w_sb[:, kc, 128 * m:128 * (m + 1)],
                                rhs=xT_sb[:, kc, 0:QC],
                                start=(kc == 0),
                                stop=(kc == KC - 1),
                            )
                    nc.vector.tensor_copy(qT_sb[:, m, 0:QC], ps[:, 0, :])
                    nc.vector.tensor_copy(kT_sb[:, m, 0:QC], ps[:, 1, :])

                def proj_v(st, pool=None):
                    ps = (pool or pflt).tile([128, QC], f32,
                                             tag="av" if pool else "flt")
                    for kc in range(KC):
                        nc.tensor.matmul(
                            ps[:, 0:GW],
                            lhsT=xT_sb[:, kc, 128 * st:128 * (st + 1)],
                            rhs=wv_sb[:, kc, :],
                            start=(kc == 0),
                            stop=(kc == KC - 1),
                        )
                    pv = ps[:, 0:GW].rearrange("p (h2 par w) -> p h2 par w",
                                               par=2, w=DK)
                    nc.vector.tensor_copy(vpar[:, st, :, 0, 0:DK], pv[:, :, 0, :])
                    nc.vector.tensor_copy(vpar[:, st, :, 1, DK:128], pv[:, :, 1, :])

                def wo_unit(c2, dm, po=None):
                    # out^T chunk [128 rows dm, 512 cols c2]
                    if po is None:
                        po = pflt.tile([128, QC], f32, tag="flt")
                    for f in range(2):
                        nc.tensor.matmul(
                            po,
                            lhsT=wo_sb[:, f, 128 * dm:128 * (dm + 1)],
                            rhs=attnT[:, f, QC * c2:QC * (c2 + 1)],
                            start=(f == 0),
                            stop=(f == 1),
                        )
                    ob = opool.tile([128, QC], fp16, tag="ob")
                    nc.vector.tensor_copy(ob, po)
                    nc.sync.dma_start(
                        outT_d.ap()[128 * dm:128 * (dm + 1),
                                    QC * c2:QC * (c2 + 1)],
                        ob,
                    )

                # ---- PE filler machinery ----
                # generators that emit one PE micro-op (one matmul) per
                # next(); interleaved between attention j-steps so the
                # FIFO-ordered PE stream has ready work in every exp-wait
                # gap. Tile deps keep any interleave correct.
                from collections import deque
                fq = deque()   # keys, in priority order
                gens = {}      # key -> generator

                def push(key, g):
                    gens[key] = g
                    fq.append(key)

                def fill(n):
                    done = 0
                    while done < n and fq:
                        k = fq[0]
                        g = gens.get(k)
                        if g is None:
                            fq.popleft()
                            continue
                        try:
                            next(g)
                            done += 1
                        except StopIteration:
                            del gens[k]
                            fq.popleft()

                def need(*keys):
                    # drain specific generators fully: a consumer is about
                    # to be emitted, its inputs must be written first in
                    # program order or no RAW dep is created
                    for k in keys:
                        g = gens.pop(k, None)
                        if g is None:
                            continue
                        for _ in g:
                            pass

                def flush():
                    while fq or gens:
                        if not fq:
                            need(*list(gens))
                            break
                        fill(64)

                def g_qk(dst_i, m, c2):
                    w_sb, dst = ((wq_sb, qT_sb), (wk_sb, kT_sb))[dst_i]
                    ps = pflt.tile([128, QC], f32, tag="flt")
                    for kc in range(KC):
                        nc.tensor.matmul(
                            ps,
                            lhsT=w_sb[:, kc, 128 * m:128 * (m + 1)],
                            rhs=xT_sb[:, kc, QC * c2:QC * (c2 + 1)],
                            start=(kc == 0),
                            stop=(kc == KC - 1),
                        )
                        if kc == KC - 1:
                            nc.vector.tensor_copy(
                                dst[:, m, QC * c2:QC * (c2 + 1)], ps)
                        yield

                def g_v(st):
                    ps = pflt.tile([128, QC], f32, tag="flt")
                    for kc in range(KC):
                        nc.tensor.matmul(
                            ps[:, 0:GW],
                            lhsT=xT_sb[:, kc, 128 * st:128 * (st + 1)],
                            rhs=wv_sb[:, kc, :],
                            start=(kc == 0),
                            stop=(kc == KC - 1),
                        )
                        if kc == KC - 1:
                            pv = ps[:, 0:GW].rearrange(
                                "p (h2 par w) -> p h2 par w", par=2, w=DK)
                            nc.vector.tensor_copy(
                                vpar[:, st, :, 0, 0:DK], pv[:, :, 0, :])
                            nc.vector.tensor_copy(
                                vpar[:, st, :, 1, DK:128], pv[:, :, 1, :])
                        yield

                def g_wo(c2, dm):
                    po = pflt.tile([128, QC], f32, tag="flt")
                    for f in range(2):
                        nc.tensor.matmul(
                            po,
                            lhsT=wo_sb[:, f, 128 * dm:128 * (dm + 1)],
                            rhs=attnT[:, f, QC * c2:QC * (c2 + 1)],
                            start=(f == 0),
                            stop=(f == 1),
                        )
                        if f == 1:
                            ob = opool.tile([128, QC], fp16, tag="ob")
                            nc.vector.tensor_copy(ob, po)
                            nc.sync.dma_start(
                                outT_d.ap()[128 * dm:128 * (dm + 1),
                                            QC * c2:QC * (c2 + 1)],
                                ob,
                            )
                        yield

                def attention(mi, c):
                    # both heads of pair mi, q-chunk c (512 wide)
                    q0 = QC * c
                    njt = (q0 + QC) // KT
                    av_e = psav.tile([128, QC], f32, tag="av", name="av_e")
                    av_o = psav.tile([128, QC], f32, tag="av", name="av_o")
                    avs = (av_e, av_o)
                    es = {}

                    def sc(j):
                        k0 = KT * j
                        vs = max(0, k0 - q0)
                        diag = k0 >= q0
                        ps = psmm.tile([128, 2, QC], f32, tag="mm")
                        for hh in range(2):
                            pb = 64 * hh
                            nc.tensor.matmul(
                                ps[:, hh, vs:QC],
                                lhsT=kT_sb[pb:pb + DK, mi, k0:k0 + KT],
                                rhs=qT_sb[pb:pb + DK, mi, q0 + vs:q0 + QC],
                                start=True,
                                stop=not diag,
                            )
                            if diag:  # staircase causal mask on diag block
                                nc.tensor.matmul(
                                    ps[:, hh, vs:vs + KT],
                                    lhsT=stA,
                                    rhs=stB,
                                    start=False,
                                    stop=True,
                                )
                        e = epool.tile([128, 2, QC], fp16, tag="e")
                        nc.scalar.activation(
                            e[:, :, vs:QC], ps[:, :, vs:QC], Exp, scale=0.125
                        )
                        es[j] = e

                    sc(0)
                    for j in range(njt):
                        if j + 1 < njt:
                            sc(j + 1)  # lookahead: exp(j) runs under sc(j+1)
                        vs = max(0, KT * j - q0)
                        e = es.pop(j)
                        for hh in range(2):
                            h = 2 * mi + hh
                            nc.tensor.matmul(
                                avs[hh][:, vs:QC],
                                lhsT=v_sb[:, j, h * 128:(h + 1) * 128],
                                rhs=e[:, hh, vs:QC],
                                start=(j == 0),
                                stop=(j == njt - 1),
                            )
                        fill(2)
                    fill(6)  # cover the reciprocal latency before bc
                    # 1/den in fp16 (|rel err| ~ 5e-4, irrelevant); PE
                    # broadcast needs both matmul operands same dtype
                    rden = npool.tile([DK + 1, QC], fp16, tag="rden")
                    with nc.allow_low_precision(reason="fp16 1/den, err ~5e-4"):
                        nc.vector.reciprocal(rden[DK:DK + 1, :], av_e[DK:DK + 1, :])
                        nc.vector.reciprocal(rden[32:33, :], av_o[32:33, :])
                    bcps = psmm.tile([128, 2, QC], f32, tag="mm")
                    bc = bcps[:, 0, :]
                    nc.tensor.matmul(
                        bc[0:DK, :],
                        lhsT=sel_sb[DK:DK + 1, 0:DK],
                        rhs=rden[DK:DK + 1, :],
                        start=True,
                        stop=True,
                    )
                    nc.tensor.matmul(
                        bc[DK:128, :],
                        lhsT=sel_sb[32:33, 0:DK],
                        rhs=rden[32:33, :],
                        start=True,
                        stop=True,
                    )
                    # tensor_tensor cannot take two PSUM inputs; stage the
                    # broadcast in SBUF once, shared by both heads
                    bc_sb = npool.tile([128, QC], f32, tag="bcsb")
                    nc.vector.tensor_copy(bc_sb, bc)
                    nc.vector.tensor_mul(
                        attnT[0:DK, mi, q0:q0 + QC], av_e[0:DK, :],
                        bc_sb[0:DK, :]
                    )
                    nc.vector.tensor_mul(
                        attnT[DK:128, mi, q0:q0 + QC], av_o[DK:128, :],
                        bc_sb[DK:128, :]
                    )

                # ---- emission order ----
                # minimal prefix before attention(0,0); projections, wo
                # chunks and later-chunk prereqs are emitted between
                # attention units so the tile scheduler can fill PE idle
                # while attention waits on ScalarE exp.
                #
                # The prefix is DMA-gated (xT chunks arrive over ~16us), so
                # every prefix unit borrows an idle psum bank: 2 psmm tiles
                # (q+k pairs), 3 psav bufs (v0-2), 1 pflt (v3) -- all 8
                # banks hold a concurrent prefix unit.
                proj_qk2(0)
                proj_qk2(1)
                for st in range(3):
                    proj_v(st, pool=psav)
                proj_v(3)

                # filler windows: each attention unit consumes queued filler
                # micro-ops; need() guarantees a unit's inputs are fully
                # emitted before the unit reads them
                def push_qk(m, c2):
                    push(("q", m, c2), g_qk(0, m, c2))
                    push(("k", m, c2), g_qk(1, m, c2))

                def need_att(mi, c):
                    need(("q", mi, c), ("k", mi, c),
                         *[("v", st) for st in range(4 * (c + 1))])

                push_qk(0, 1)
                for st in range(4, 8):
                    push(("v", st), g_v(st))
                push_qk(1, 1)
                attention(0, 0)
                attention(1, 0)

                for dm in range(8):
                    push(("wo", 0, dm), g_wo(0, dm))
                push_qk(0, 2)
                for st in range(8, 12):
                    push(("v", st), g_v(st))
                push_qk(1, 2)
                need_att(0, 1)
                attention(0, 1)
                need_att(1, 1)
                attention(1, 1)

                for dm in range(8):
                    push(("wo", 1, dm), g_wo(1, dm))
                push_qk(0, 3)
                for st in range(12, 16):
                    push(("v", st), g_v(st))
                push_qk(1, 3)
                need_att(0, 2)
                attention(0, 2)
                need_att(1, 2)
                attention(1, 2)

                for dm in range(8):
                    push(("wo", 2, dm), g_wo(2, dm))
                need_att(0, 3)
                attention(0, 3)
                need_att(1, 3)
                attention(1, 3)
                flush()
                # tail: attention is done, all psum pools are free -- run
                # the last wo chunk wide through psmm/psav tiles
                po2 = psmm.tile([128, 2, QC], f32, tag="mm")
                po3 = psmm.tile([128, 2, QC], f32, tag="mm")
                wo_unit(3, 0, po=po2[:, 0, :])
                wo_unit(3, 1, po=po2[:, 1, :])
                wo_unit(3, 2, po=po3[:, 0, :])
                wo_unit(3, 3, po=po3[:, 1, :])
                for i, dm in enumerate(range(4, 7)):
                    po4 = psav.tile([128, QC], f32, tag="av", name=f"wo3_{dm}")
                    wo_unit(3, dm, po=po4)
                wo_unit(3, 7)

    nc.compile()
    return nc


def _get_nc():
    if "nc" not in _CACHE:
        _CACHE["nc"] = _build_nc()
    return _CACHE["nc"]


def _stairs():
    t = np.arange(128)
    stA = (t[:, None] <= t[None, :]).astype(np.float16)
    stB = np.where(t[:, None] > t[None, :], MASK_STEP, 0.0).astype(np.float16)
    return stA, stB


def _sel():
    # ones rows for the 1/den broadcast matmuls: row 64 (even-head den)
    # and row 32 (odd-head den); lhsT free size 64 -> 64 out partitions
    sel = np.zeros((65, 128), np.float16)
    sel[64, 0:64] = 1.0
    sel[32, 0:64] = 1.0
    return sel


def _make_in_maps(x, wq, wk, wv, wo):
    stA, stB = _stairs()
    sel = _sel()
    x = np.asarray(x, np.float32)
    xTs = [np.ascontiguousarray(x[b].T).astype(np.float16) for b in range(B)]
    wqh = np.asarray(wq, np.float32).astype(np.float16)
    wkh = np.asarray(wk, np.float32).astype(np.float16)
    wvh = np.asarray(wv, np.float32).astype(np.float16)
    woh = np.asarray(wo, np.float32).astype(np.float16)
    in_maps = []
    for c in range(NCORES):
        b, g = divmod(c, HPC)
        cols = slice(g * GW, (g + 1) * GW)
        in_maps.append({
            "xT": xTs[b],
            "wq": np.ascontiguousarray(wqh[:, cols]),
            "wk": np.ascontiguousarray(wkh[:, cols]),
            "wv": np.ascontiguousarray(wvh[:, cols]),
            "wo": np.ascontiguousarray(woh[cols, :]),
            "stairA": stA,
            "stairB": stB,
            "sel": sel,
        })
    return in_maps


def run(x, wq, wk, wv, wo, trace=False):
    from concourse.bass_utils import run_bass_kernel_spmd

    nc = _get_nc()
    in_maps = _make_in_maps(x, wq, wk, wv, wo)
    res = run_bass_kernel_spmd(nc, in_maps, list(range(NCORES)), trace=trace)
    acc = np.zeros((B, D, S), np.float64)
    for c in range(NCORES):
        acc[c // HPC] += res.results[c]["outT"]
    out = np.ascontiguousarray(acc.transpose(0, 2, 1).astype(np.float32))
    return out, res


def kernel(x, wq, wk, wv, wo):
    out, _ = run(x, wq, wk, wv, wo, trace=False)
    return out


# revision 27
# speedup vs baseline: 1.1432x; 1.0262x over previous
"""Causal multi-head self-attention on 8 Trainium2 NeuronCores.

Problem: x[2,2048,1024], 16 heads, dk=64, causal softmax, fp32 in/out.

Sharding (data + tensor parallel, per the hint): core c handles batch
b = c//4 and head group g = c%4 (4 heads = 256 feature cols). wq/wk/wv
are column-sharded, wo row-sharded; each core returns a [D, S] fp16
partial of out^T for its batch, and the host sums the 4 partials per
batch in fp64.

Per-core kernel, fp16 operands (matmul accumulation fp32 in PSUM):
  - host supplies x^T [D, S]; q^T/k^T [256, S] = w^T @ x^T on PE,
    v [S, 256] natural, all computed on-device in fp16.
  - attention runs per head pair mi and 512-wide q-chunk c. Both heads
    of a pair share one scores psum tile [128, 2, 512] (one 2KB bank
    per head), so a single ScalarE exp (scale=1/8 fused, no
    max-subtraction: |scores/8|<~3; masked entries underflow to 0)
    covers both heads: e[128, 2, 512] fp16.
  - causal masking: k-tiles strictly above the diagonal are skipped;
    the diagonal 128x128 block gets a staircase additive mask
    (-240*(k-q) for k>q) from one extra fp16 matmul.
  - v is stored per head in 128-wide groups with parity layout:
    even heads [v(cols 0:64) | ones@64 | 0], odd heads
    [0 | ones@63 | v(cols 64:128)]. The AV matmul then lands even-head
    attn on psum rows 0:64 (denominator row 64) and odd-head attn on
    rows 64:128 (denominator row 63) -- partition-aligned with the
    attnT layout the wo matmul wants, so no cross-partition moves.
  - softmax normalize: DVE reciprocal on the two den rows (63, 64),
    then one K=2 PE matmul against a constant selector broadcasts
    1/den_even to partitions 0:64 and 1/den_odd to 64:128 in one psum
    tile (rhs dtype float32r: full fp32 data, 1 cycle/row). Two DVE
    tensor_muls write normalized attnT [128, 2, S] fp16.
  - out^T partial [D, S] fp16 = wo.T @ attnT on PE in [128,512] psum
    chunks; the f=0 (heads 0,1) contraction half is emitted as soon as
    pair 0's attnT chunk is ready so wo overlaps attention.
  - emission order interleaves projection / wo chunks between
    attention units as PE filler (attention is ScalarE-exp paced); a
    dedicated 2-buf [128,512] psum pool serves proj/wo/bc so fillers
    never contend with the scores/av psum pools.
"""

import os
import sys

import numpy as np

if "/opt/trn_rl_repo" not in sys.path:
    sys.path.insert(0, "/opt/trn_rl_repo")

DEBUG = bool(os.environ.get("BASSDBG"))

B, S, D, H, DK = 2, 2048, 1024, 16, 64
HPC = 4            # heads per core
GW = HPC * DK      # 256
NCORES = 8
QC = 512           # q-chunk width (1 psum bank of fp32)
NQC = S // QC      # 4
KT = 128           # k-tile
MASK_STEP = -240.0

_CACHE = {}


def _build_nc(reps=1):
    import concourse.bacc as bacc
    import concourse.tile as tile
    import concourse.bass as bass
    from concourse import mybir

    f32 = mybir.dt.float32
    f32r = mybir.dt.float32r
    fp16 = mybir.dt.float16
    Exp = mybir.ActivationFunctionType.Exp
    PSUM = bass.MemorySpace.PSUM

    nc = bacc.Bacc(
        "TRN2",
        target_bir_lowering=False,
        debug=False,
        enable_asserts=False,
        num_devices=NCORES,
    )

    xT_d = nc.dram_tensor("xT", [D, S], fp16, kind="ExternalInput")
    wq_d = nc.dram_tensor("wq", [D, GW], fp16, kind="ExternalInput")
    wk_d = nc.dram_tensor("wk", [D, GW], fp16, kind="ExternalInput")
    wv_d = nc.dram_tensor("wv", [D, GW], fp16, kind="ExternalInput")
    wo_d = nc.dram_tensor("wo", [GW, D], fp16, kind="ExternalInput")
    stA_d = nc.dram_tensor("stairA", [128, 128], fp16, kind="ExternalInput")
    stB_d = nc.dram_tensor("stairB", [128, 128], fp16, kind="ExternalInput")
    sel_d = nc.dram_tensor("sel", [65, 128], fp16, kind="ExternalInput")
    outT_d = nc.dram_tensor("outT", [D, S], fp16, kind="ExternalOutput")

    KC = D // 128  # 8 contraction chunks for the projections

    with tile.TileContext(nc) as tc:
        with (
            tc.tile_pool(name="weights", bufs=1) as wpool,
            tc.tile_pool(name="acts", bufs=1) as apool,
            tc.tile_pool(name="psmm", bufs=2, space=PSUM) as psmm,
            tc.tile_pool(name="psav", bufs=2, space=PSUM) as psav,
            tc.tile_pool(name="pflt", bufs=2, space=PSUM) as pflt,
            tc.tile_pool(name="epool", bufs=8) as epool,
            tc.tile_pool(name="norm", bufs=3) as npool,
            tc.tile_pool(name="outp", bufs=4) as opool,
        ):
            # ---- loads ----
            stA = wpool.tile([128, 128], fp16, tag="stA")
            stB = wpool.tile([128, 128], fp16, tag="stB")
            sel_sb = wpool.tile([65, 128], fp16, tag="sel")
            wq_sb = wpool.tile([128, KC, GW], fp16, tag="wq")
            wk_sb = wpool.tile([128, KC, GW], fp16, tag="wk")
            wv_sb = wpool.tile([128, KC, GW], fp16, tag="wv")
            wo_sb = wpool.tile([128, 2, D], fp16, tag="wo")
            nc.sync.dma_start(wq_sb, wq_d.ap().rearrange("(kc p) m -> p kc m", p=128))

            first_rep = True
            for _rep in range(reps):  # >1 only for timing builds
                xT_sb = apool.tile([128, KC, S], fp16, tag="xT", name=f"xT_sb{_rep}")
                xT_view = xT_d.ap().rearrange("(kc p) s -> p kc s", p=128)
                nc.sync.dma_start(xT_sb[:, 0, :], xT_view[:, 0, :])
                if first_rep:
                    first_rep = False
                    nc.sync.dma_start(
                        wk_sb, wk_d.ap().rearrange("(kc p) m -> p kc m", p=128))
                    nc.sync.dma_start(
                        wv_sb, wv_d.ap().rearrange("(kc p) m -> p kc m", p=128))
                    nc.sync.dma_start(stA, stA_d.ap())
                    nc.sync.dma_start(stB, stB_d.ap())
                    nc.sync.dma_start(sel_sb, sel_d.ap())
                for kc in range(1, KC):
                    nc.sync.dma_start(xT_sb[:, kc, :], xT_view[:, kc, :])
                if _rep == 0:
                    nc.sync.dma_start(
                        wo_sb, wo_d.ap().rearrange("(f p) n -> p f n", p=128))

                qT_sb = apool.tile([128, 2, S], fp16, tag="qT")
                kT_sb = apool.tile([128, 2, S], fp16, tag="kT")
                # v per head in 128-wide groups, parity layout (see module
                # docstring); den rows land at psum partitions 64 (even
                # head) and 63 (odd head)
                v_sb = apool.tile([128, S // 128, HPC * 128], fp16, tag="v")
                vpar = v_sb.rearrange("p st (h2 par w) -> p st h2 par w",
                                      par=2, w=128)
                # even head: v @ cols 0:64, ones col 64 -> den row 64
                # odd head: ones col 32 -> den row 32, v @ cols 64:128
                # (matmul operand bases must be 0/32/64)
                nc.gpsimd.memset(vpar[:, :, :, 0, DK + 1:128], 0.0)
                nc.gpsimd.memset(vpar[:, :, :, 1, 0:32], 0.0)
                nc.gpsimd.memset(vpar[:, :, :, 1, 33:DK], 0.0)
                nc.gpsimd.memset(vpar[:, :, :, 0, DK:DK + 1], 1.0)
                nc.gpsimd.memset(vpar[:, :, :, 1, 32:33], 1.0)
                attnT = apool.tile([128, 2, S], fp16, tag="attnT")

                def proj_qk(dst_i, m, c2):
                    # q (dst_i=0) or k (dst_i=1), pair m, S-chunk c2 (512)
                    w_sb, dst = ((wq_sb, qT_sb), (wk_sb, kT_sb))[dst_i]
                    ps = pflt.tile([128, QC], f32, tag="flt")
                    for kc in range(KC):
                        nc.tensor.matmul(
                            ps,
                            lhsT=w_sb[:, kc, 128 * m:128 * (m + 1)],
                            rhs=xT_sb[:, kc, QC * c2:QC * (c2 + 1)],
                            start=(kc == 0),
                            stop=(kc == KC - 1),
                        )
                    nc.vector.tensor_copy(dst[:, m, QC * c2:QC * (c2 + 1)], ps)

                def proj_qk2(m):
                    # startup variant: q and k for chunk 0 share one psmm
                    # tile (2 banks), interleaved per xT chunk arrival so PE
                    # is never gated on the 1-buf filler pool during load
                    ps = psmm.tile([128, 2, QC], f32, tag="mm")
                    for kc in range(KC):
                        for di, w_sb in ((0, wq_sb), (1, wk_sb)):
                            nc.tensor.matmul(
                                ps[:, di, :],
                                lhsT=w_sb[:, kc, 128 * m:128 * (m + 1)],
                                rhs=xT_sb[:, kc, 0:QC],
                                start=(kc == 0),
                                stop=(kc == KC - 1),
                            )
                    nc.vector.tensor_copy(qT_sb[:, m, 0:QC], ps[:, 0, :])
                    nc.vector.tensor_copy(kT_sb[:, m, 0:QC], ps[:, 1, :])

                def proj_v(st, pool=None):
                    ps = (pool or pflt).tile([128, QC], f32,
                                             tag="av" if pool else "flt")
                    for kc in range(KC):
                        nc.tensor.matmul(
                            ps[:, 0:GW],
                            lhsT=xT_sb[:, kc, 128 * st:128 * (st + 1)],
                            rhs=wv_sb[:, kc, :],
                            start=(kc == 0),
                            stop=(kc == KC - 1),
                        )
                    pv = ps[:, 0:GW].rearrange("p (h2 par w) -> p h2 par w",
                                               par=2, w=DK)
                    nc.vector.tensor_copy(vpar[:, st, :, 0, 0:DK], pv[:, :, 0, :])
                    nc.vector.tensor_copy(vpar[:, st, :, 1, DK:128], pv[:, :, 1, :])

                def wo_unit(c2, dm, po=None):
                    # out^T chunk [128 rows dm, 512 cols c2]
                    if po is None:
                        po = pflt.tile([128, QC], f32, tag="flt")
                    for f in range(2):
                        nc.tensor.matmul(
                            po,
                            lhsT=wo_sb[:, f, 128 * dm:128 * (dm + 1)],
                            rhs=attnT[:, f, QC * c2:QC * (c2 + 1)],
                            start=(f == 0),
                            stop=(f == 1),
                        )
                    ob = opool.tile([128, QC], fp16, tag="ob")
                    nc.vector.tensor_copy(ob, po)
                    nc.sync.dma_start(
                        outT_d.ap()[128 * dm:128 * (dm + 1),
                                    QC * c2:QC * (c2 + 1)],
                        ob,
                    )

                # ---- PE filler machinery ----
                # generators that emit one PE micro-op (one matmul) per
                # next(); interleaved between attention j-steps so the
                # FIFO-ordered PE stream has ready work in every exp-wait
                # gap. Tile deps keep any interleave correct.
                from collections import deque
                fq = deque()   # keys, in priority order
                gens = {}      # key -> generator

                def push(key, g):
                    gens[key] = g
                    fq.append(key)

                def fill(n):
                    done = 0
                    while done < n and fq:
                        k = fq[0]
                        g = gens.get(k)
                        if g is None:
                            fq.popleft()
                            continue
                        try:
                            next(g)
                            done += 1
                        except StopIteration:
                            del gens[k]
                            fq.popleft()

                def need(*keys):
                    # drain specific generators fully: a consumer is about
                    # to be emitted, its inputs must be written first in
                    # program order or no RAW dep is created
                    for k in keys:
                        g = gens.pop(k, None)
                        if g is None:
                            continue
                        for _ in g:
                            pass

                def flush():
                    while fq or gens:
                        if not fq:
                            need(*list(gens))
                            break
                        fill(64)

                def g_qk(dst_i, m, c2):
                    w_sb, dst = ((wq_sb, qT_sb), (wk_sb, kT_sb))[dst_i]
                    ps = pflt.tile([128, QC], f32, tag="flt")
                    for kc in range(KC):
                        nc.tensor.matmul(
                            ps,
                            lhsT=w_sb[:, kc, 128 * m:128 * (m + 1)],
                            rhs=xT_sb[:, kc, QC * c2:QC * (c2 + 1)],
                            start=(kc == 0),
                            stop=(kc == KC - 1),
                        )
                        if kc == KC - 1:
                            nc.vector.tensor_copy(
                                dst[:, m, QC * c2:QC * (c2 + 1)], ps)
                        yield

                def g_v(st):
                    ps = pflt.tile([128, QC], f32, tag="flt")
                    for kc in range(KC):
                        nc.tensor.matmul(
                            ps[:, 0:GW],
                            lhsT=xT_sb[:, kc, 128 * st:128 * (st + 1)],
                            rhs=wv_sb[:, kc, :],
                            start=(kc == 0),
                            stop=(kc == KC - 1),
                        )
                        if kc == KC - 1:
                            pv = ps[:, 0:GW].rearrange(
                                "p (h2 par w) -> p h2 par w", par=2, w=DK)
                            nc.vector.tensor_copy(
                                vpar[:, st, :, 0, 0:DK], pv[:, :, 0, :])
                            nc.vector.tensor_copy(
                                vpar[:, st, :, 1, DK:128], pv[:, :, 1, :])
                        yield

                def g_wo(c2, dm):
                    po = pflt.tile([128, QC], f32, tag="flt")
                    for f in range(2):
                        nc.tensor.matmul(
                            po,
                            lhsT=wo_sb[:, f, 128 * dm:128 * (dm + 1)],
                            rhs=attnT[:, f, QC * c2:QC * (c2 + 1)],
                            start=(f == 0),
                            stop=(f == 1),
                        )
                        if f == 1:
                            ob = opool.tile([128, QC], fp16, tag="ob")
                            nc.vector.tensor_copy(ob, po)
                            nc.sync.dma_start(
                                outT_d.ap()[128 * dm:128 * (dm + 1),
                                            QC * c2:QC * (c2 + 1)],
                                ob,
                            )
                        yield

                def attention(mi, c):
                    # both heads of pair mi, q-chunk c (512 wide)
                    q0 = QC * c
                    njt = (q0 + QC) // KT
                    av_e = psav.tile([128, QC], f32, tag="av", name="av_e")
                    av_o = psav.tile([128, QC], f32, tag="av", name="av_o")
                    avs = (av_e, av_o)
                    es = {}

                    def sc(j):
                        k0 = KT * j
                        vs = max(0, k0 - q0)
                        diag = k0 >= q0
                        ps = psmm.tile([128, 2, QC], f32, tag="mm")
                        for hh in range(2):
                            pb = 64 * hh
                            nc.tensor.matmul(
                                ps[:, hh, vs:QC],
                                lhsT=kT_sb[pb:pb + DK, mi, k0:k0 + KT],
                                rhs=qT_sb[pb:pb + DK, mi, q0 + vs:q0 + QC],
                                start=True,
                                stop=not diag,
                            )
                            if diag:  # staircase causal mask on diag block
                                nc.tensor.matmul(
                                    ps[:, hh, vs:vs + KT],
                                    lhsT=stA,
                                    rhs=stB,
                                    start=False,
                                    stop=True,
                                )
                        e = epool.tile([128, 2, QC], fp16, tag="e")
                        nc.scalar.activation(
                            e[:, :, vs:QC], ps[:, :, vs:QC], Exp, scale=0.125
                        )
                        es[j] = e

                    sc(0)
                    for j in range(njt):
                        if j + 1 < njt:
                            sc(j + 1)  # lookahead: exp(j) runs under sc(j+1)
                        vs = max(0, KT * j - q0)
                        e = es.pop(j)
                        for hh in range(2):
                            h = 2 * mi + hh
                            nc.tensor.matmul(
                                avs[hh][:, vs:QC],
                                lhsT=v_sb[:, j, h * 128:(h + 1) * 128],
                                rhs=e[:, hh, vs:QC],
                                start=(j == 0),
                                stop=(j == njt - 1),
                            )
                        fill(2)
                    fill(6)  # cover the reciprocal latency before bc
                    # 1/den in fp16 (|rel err| ~ 5e-4, irrelevant); PE
                    # broadcast needs both matmul operands same dtype
                    rden = npool.tile([DK + 1, QC], fp16, tag="rden")
                    with nc.allow_low_precision(reason="fp16 1/den, err ~5e-4"):
                        nc.vector.reciprocal(rden[DK:DK + 1, :], av_e[DK:DK + 1, :])
                        nc.vector.reciprocal(rden[32:33, :], av_o[32:33, :])
                    bcps = psmm.tile([128, 2, QC], f32, tag="mm")
                    bc = bcps[:, 0, :]
                    nc.tensor.matmul(
                        bc[0:DK, :],
                        lhsT=sel_sb[DK:DK + 1, 0:DK],
                        rhs=rden[DK:DK + 1, :],
                        start=True,
                        stop=True,
                    )
                    nc.tensor.matmul(
                        bc[DK:128, :],
                        lhsT=sel_sb[32:33, 0:DK],
                        rhs=rden[32:33, :],
                        start=True,
                        stop=True,
                    )
                    # tensor_tensor cannot take two PSUM inputs; stage the
                    # broadcast in SBUF once, shared by both heads
                    bc_sb = npool.tile([128, QC], f32, tag="bcsb")
                    nc.vector.tensor_copy(bc_sb, bc)
                    nc.vector.tensor_mul(
                        attnT[0:DK, mi, q0:q0 + QC], av_e[0:DK, :],
                        bc_sb[0:DK, :]
                    )
                    nc.vector.tensor_mul(
                        attnT[DK:128, mi, q0:q0 + QC], av_o[DK:128, :],
                        bc_sb[DK:128, :]
                    )

                # ---- emission order ----
                # minimal prefix before attention(0,0); projections, wo
                # chunks and later-chunk prereqs are emitted between
                # attention units so the tile scheduler can fill PE idle
                # while attention waits on ScalarE exp.
                #
                # The prefix is DMA-gated (xT chunks arrive over ~16us):
                # emit chunk-major so each arriving xT chunk unlocks all 8
                # units' matmuls back-to-back, with every psum bank holding
                # one concurrent prefix unit (2 psmm pairs + 2 psav + 2 pflt).
                pqk = [psmm.tile([128, 2, QC], f32, tag="mm", name=f"pqk{m}")
                       for m in range(2)]
                pv = [psav.tile([128, QC], f32, tag="av", name="pv0"),
                      psav.tile([128, QC], f32, tag="av", name="pv1"),
                      pflt.tile([128, QC], f32, tag="flt", name="pv2"),
                      pflt.tile([128, QC], f32, tag="flt", name="pv3")]
                for kc in range(KC):
                    st_flags = dict(start=(kc == 0), stop=(kc == KC - 1))
                    for m in range(2):
                        for di, w_sb in ((0, wq_sb), (1, wk_sb)):
                            nc.tensor.matmul(
                                pqk[m][:, di, :],
                                lhsT=w_sb[:, kc, 128 * m:128 * (m + 1)],
                                rhs=xT_sb[:, kc, 0:QC],
                                **st_flags,
                            )
                    for st in range(4):
                        nc.tensor.matmul(
                            pv[st][:, 0:GW],
                            lhsT=xT_sb[:, kc, 128 * st:128 * (st + 1)],
                            rhs=wv_sb[:, kc, :],
                            **st_flags,
                        )
                for m in range(2):
                    nc.vector.tensor_copy(qT_sb[:, m, 0:QC], pqk[m][:, 0, :])
                    nc.vector.tensor_copy(kT_sb[:, m, 0:QC], pqk[m][:, 1, :])
                for st in range(4):
                    pvv = pv[st][:, 0:GW].rearrange(
                        "p (h2 par w) -> p h2 par w", par=2, w=DK)
                    nc.vector.tensor_copy(vpar[:, st, :, 0, 0:DK],
                                          pvv[:, :, 0, :])
                    nc.vector.tensor_copy(vpar[:, st, :, 1, DK:128],
                                          pvv[:, :, 1, :])

                # filler windows: each attention unit consumes queued filler
                # micro-ops; need() guarantees a unit's inputs are fully
                # emitted before the unit reads them
                def push_qk(m, c2):
                    push(("q", m, c2), g_qk(0, m, c2))
                    push(("k", m, c2), g_qk(1, m, c2))

                def need_att(mi, c):
                    need(("q", mi, c), ("k", mi, c),
                         *[("v", st) for st in range(4 * (c + 1))])

                push_qk(0, 1)
                for st in range(4, 8):
                    push(("v", st), g_v(st))
                push_qk(1, 1)
                attention(0, 0)
                attention(1, 0)

                for dm in range(8):
                    push(("wo", 0, dm), g_wo(0, dm))
                push_qk(0, 2)
                for st in range(8, 12):
                    push(("v", st), g_v(st))
                push_qk(1, 2)
                need_att(0, 1)
                attention(0, 1)
                need_att(1, 1)
                attention(1, 1)

                for dm in range(8):
                    push(("wo", 1, dm), g_wo(1, dm))
                push_qk(0, 3)
                for st in range(12, 16):
                    push(("v", st), g_v(st))
                push_qk(1, 3)
                need_att(0, 2)
                attention(0, 2)
                need_att(1, 2)
                attention(1, 2)

                for dm in range(8):
                    push(("wo", 2, dm), g_wo(2, dm))
                need_att(0, 3)
                attention(0, 3)
                need_att(1, 3)
                attention(1, 3)
                flush()
                # tail: attention is done, all psum pools are free -- run
                # the last wo chunk wide through psmm/psav tiles
                po2 = psmm.tile([128, 2, QC], f32, tag="mm")
                po3 = psmm.tile([128, 2, QC], f32, tag="mm")
                wo_unit(3, 0, po=po2[:, 0, :])
                wo_unit(3, 1, po=po2[:, 1, :])
                wo_unit(3, 2, po=po3[:, 0, :])
                wo_unit(3, 3, po=po3[:, 1, :])
                for dm in range(4, 6):
                    po4 = psav.tile([128, QC], f32, tag="av", name=f"wo3_{dm}")
                    wo_unit(3, dm, po=po4)
                wo_unit(3, 6)
                wo_unit(3, 7)

    nc.compile()
    return nc


def _get_nc():
    if "nc" not in _CACHE:
        _CACHE["nc"] = _build_nc()
    return _CACHE["nc"]


def _stairs():
    t = np.arange(128)
    stA = (t[:, None] <= t[None, :]).astype(np.float16)
    stB = np.where(t[:, None] > t[None, :], MASK_STEP, 0.0).astype(np.float16)
    return stA, stB


def _sel():
    # ones rows for the 1/den broadcast matmuls: row 64 (even-head den)
    # and row 32 (odd-head den); lhsT free size 64 -> 64 out partitions
    sel = np.zeros((65, 128), np.float16)
    sel[64, 0:64] = 1.0
    sel[32, 0:64] = 1.0
    return sel


def _make_in_maps(x, wq, wk, wv, wo):
    stA, stB = _stairs()
    sel = _sel()
    x = np.asarray(x, np.float32)
    xTs = [np.ascontiguousarray(x[b].T).astype(np.float16) for b in range(B)]
    wqh = np.asarray(wq, np.float32).astype(np.float16)
    wkh = np.asarray(wk, np.float32).astype(np.float16)
    wvh = np.asarray(wv, np.float32).astype(np.float16)
    woh = np.asarray(wo, np.float32).astype(np.float16)
    in_maps = []
    for c in range(NCORES):
        b, g = divmod(c, HPC)
        cols = slice(g * GW, (g + 1) * GW)
        in_maps.append({
            "xT": xTs[b],
            "wq": np.ascontiguousarray(wqh[:, cols]),
            "wk": np.ascontiguousarray(wkh[:, cols]),
            "wv": np.ascontiguousarray(wvh[:, cols]),
            "wo": np.ascontiguousarray(woh[cols, :]),
            "stairA": stA,
            "stairB": stB,
            "sel": sel,
        })
    return in_maps


def run(x, wq, wk, wv, wo, trace=False):
    from concourse.bass_utils import run_bass_kernel_spmd

    nc = _get_nc()
    in_maps = _make_in_maps(x, wq, wk, wv, wo)
    res = run_bass_kernel_spmd(nc, in_maps, list(range(NCORES)), trace=trace)
    acc = np.zeros((B, D, S), np.float64)
    for c in range(NCORES):
        acc[c // HPC] += res.results[c]["outT"]
    out = np.ascontiguousarray(acc.transpose(0, 2, 1).astype(np.float32))
    return out, res


def kernel(x, wq, wk, wv, wo):
    out, _ = run(x, wq, wk, wv, wo, trace=False)
    return out
